# revision 18
# baseline (speedup 1.0000x reference)
"""Trainium2 Bass kernel for a dense transformer block (B=4,T=2048,H=16,D=64,C=1024,FF=4096).

Sharding: batch b -> core pair (2b, 2b+1). Within a pair, attention is split by
heads (8 heads/core, Megatron column-parallel QKV + row-parallel W_o), the
attention output partial sums are combined with a pair ReduceScatter, and each
core then runs the full-FF MLP on its half (1024) of the rows.

v2 attention pipeline (per 512-wide q-slice, heads processed in groups of 4):
  - S^T = K @ Q^T per head pair with 64-row PE tiling: the two heads of a pair
    live at SBUF partitions 0-63 / 64-127, so their K=64 matmuls go to PE row
    tiles (0,0)/(64,0) and run concurrently.
  - exp on ScalarE (the attention bottleneck engine) -> PT bf16.
  - softmax denominators via M=1 matmuls (ones_col.T @ PT) 4-way column-tiled
    to PSUM partitions 0/32/64/96 of one bank, accumulated over key chunks.
  - One batched reciprocal per 4-head group, recip broadcast to 64 partitions
    via paired fp32 matmuls (row tiles 0/32/64/96 x col tiles 0/64).
  - AV as V^T @ PT with M=64 column-tiled pairs: head A -> PSUM 0-63, head B
    -> 64-127 of one bank; denominator folded in afterwards by one DVE
    multiply per pair (po * recb) straight into YT.
  - Fully-masked key blocks are never computed; the partially-masked diagonal
    128-block is exp'd then multiplied by a constant triangle (DVE). The
    region left of the diagonal is skipped via subrange accumulation in the
    den/AV matmuls (kc=0 always starts full-width), so no memsets.

LayerNorm affines are folded into the following matmul weights on the host.
Matmuls run in bf16 with fp32 PSUM accumulation. QKV is produced per-512-slice
(Q then K then V) so attention starts while QKV is still streaming; QKV bias
adds run on VectorE to keep ScalarE free for exp. The MLP FC loop runs
t-slice-major (palindrome over weight chunks) so the last ReduceScatter chunk
and LN2 tail hide under the first FC t-slice.
"""

import math

import ml_dtypes
import numpy as np

P = 128
B, T, H, D = 4, 2048, 16, 64
C = H * D
FF = 4096
EPS = 1e-5
N_CORES = 8

_CACHE = {}
LAST_RESULT = None


def _build(T, C, H, D, FF, n_cores, groups, phase_limit=99, sim_safe=False):
    """Build + compile the single-core SPMD program. Returns the Bacc object."""
    from contextlib import ExitStack

    import concourse.mybir as mybir
    import concourse.tile as tile
    from concourse import bacc

    dt = mybir.dt
    AF = mybir.ActivationFunctionType
    OP = mybir.AluOpType

    HH = H // 2               # heads per core
    QH = HH * D               # per-core c_out for each of q,k,v
    NQH = QH // P
    NT = T // P
    T2 = T // 2               # own rows
    NT2 = T2 // P
    NC = C // P
    NF = FF // P
    SL = min(512, T)          # attention q-slice width
    NSL = T // SL
    DBLK = SL // P
    HPC = P // D              # heads per 128-partition chunk (=2)
    FCW = min(512, FF)        # wfc col-chunk width
    TSW = min(512, T)         # qkv t-slice width
    NTS = T // TSW
    CSW = min(512, C)
    NCS = C // CSW
    TS2 = min(512, T2)
    NT2S = T2 // TS2
    NB = 4 if NC % 4 == 0 else 1  # transposes batched per psum bank
    assert QH % P == 0 and T % SL == 0 and SL % P == 0

    nc = bacc.Bacc("TRN2", target_bir_lowering=False, debug=False,
                   num_devices=n_cores)
    gelu_af = (mybir.ActivationFunctionType.Identity if sim_safe
               else mybir.ActivationFunctionType.Gelu)

    # ---- kernel I/O ----
    x_full = nc.dram_tensor("x_full", [T, C], dt.float32, kind="ExternalInput")
    x_own = nc.dram_tensor("x_own", [T2, C], dt.float32, kind="ExternalInput")
    wq = nc.dram_tensor("wq", [C, QH], dt.bfloat16, kind="ExternalInput")
    wk = nc.dram_tensor("wk", [C, QH], dt.bfloat16, kind="ExternalInput")
    wv = nc.dram_tensor("wv", [C, QH], dt.bfloat16, kind="ExternalInput")
    bq = nc.dram_tensor("bq", [QH], dt.float32, kind="ExternalInput")
    bk = nc.dram_tensor("bk", [QH], dt.float32, kind="ExternalInput")
    bv = nc.dram_tensor("bv", [QH], dt.float32, kind="ExternalInput")
    wo = nc.dram_tensor("wo", [QH, C], dt.bfloat16, kind="ExternalInput")
    bo = nc.dram_tensor("bo", [C], dt.float32, kind="ExternalInput")
    wfc = nc.dram_tensor("wfc", [C, FF], dt.bfloat16, kind="ExternalInput")
    bfc = nc.dram_tensor("bfc", [FF], dt.float32, kind="ExternalInput")
    wout = nc.dram_tensor("wout", [FF, C], dt.bfloat16, kind="ExternalInput")
    bout = nc.dram_tensor("bout", [C], dt.float32, kind="ExternalInput")
    tri = nc.dram_tensor("tri", [P, P], dt.bfloat16, kind="ExternalInput")
    ident = nc.dram_tensor("ident", [P, P], dt.bfloat16, kind="ExternalInput")
    out = nc.dram_tensor("out", [T2, C], dt.float32, kind="ExternalOutput")

    # collective bounce buffers (internal DRAM)
    r_bounce = nc.dram_tensor("r_bounce", [T, C], dt.bfloat16)
    r_own_b = nc.dram_tensor("r_own_b", [T2, C], dt.bfloat16)

    x_r = x_full.rearrange("(i p) c -> p i c", p=P)
    xo_r = x_own.rearrange("(i p) c -> p i c", p=P)
    out_r = out.rearrange("(i p) c -> p i c", p=P)
    rb_r = r_bounce.rearrange("(i p) c -> p i c", p=P)
    rob_r = r_own_b.rearrange("(i p) c -> p i c", p=P)

    with tile.TileContext(nc) as tc, ExitStack() as stk:
        pool_const = stk.enter_context(tc.tile_pool(name="const", bufs=1))

        tri_sb = pool_const.tile([P, P], dt.bfloat16)
        id_sb = pool_const.tile([P, P], dt.bfloat16)
        nc.sync.dma_start(tri_sb[:], tri[:])
        nc.sync.dma_start(id_sb[:], ident[:])
        bq_sb = pool_const.tile([P, NQH], dt.float32)
        bk_sb = pool_const.tile([P, NQH], dt.float32)
        bv_row = pool_const.tile([1, QH], dt.float32)
        bo_row = pool_const.tile([1, C], dt.float32)
        bfc_sb = pool_const.tile([P, NF], dt.float32)
        bout_row = pool_const.tile([1, C], dt.float32)
        eps_sb = pool_const.tile([P, 1], dt.float32)
        nc.vector.memset(eps_sb[:], EPS)
        ones1 = pool_const.tile([1, P], dt.float32)
        nc.vector.memset(ones1[:], 1.0)
        bv_full = pool_const.tile([P, QH], dt.bfloat16)
        bo_full = pool_const.tile([P, C], dt.bfloat16)
        bout_full = pool_const.tile([P, C], dt.bfloat16)
        nc.sync.dma_start(bq_sb[:], bq.rearrange("(a p) -> p a", p=P))
        nc.sync.dma_start(bk_sb[:], bk.rearrange("(a p) -> p a", p=P))
        nc.sync.dma_start(bv_row[:], bv[None, :])
        nc.sync.dma_start(bo_row[:], bo[None, :])
        nc.sync.dma_start(bfc_sb[:], bfc.rearrange("(a p) -> p a", p=P))
        nc.sync.dma_start(bout_row[:], bout[None, :])

        def ln_alloc(pool, n_chunks, nm):
            tiles = {}
            for t in ("s1", "s2", "mean", "var", "rstd", "nmr"):
                tiles[t] = pool.tile([P, n_chunks], dt.float32,
                                     tag=f"ln_{t}", name=f"{t}_{nm}")
            return tiles

        def ln_chunk(st, i, xc):
            # stats + rstd/-mean*rstd for one [P, C] chunk (column i)
            ii = slice(i, i + 1)
            nc.vector.reduce_sum(st["s1"][:, ii], xc,
                                 axis=mybir.AxisListType.X)
            nc.vector.tensor_scalar_mul(st["mean"][:, ii], st["s1"][:, ii],
                                        1.0 / C)
            nc.vector.tensor_scalar_mul(st["var"][:, ii], st["s2"][:, ii],
                                        1.0 / C)
            nc.vector.tensor_tensor(st["nmr"][:, ii], st["mean"][:, ii],
                                    st["mean"][:, ii], OP.mult)
            nc.vector.tensor_tensor(st["var"][:, ii], st["var"][:, ii],
                                    st["nmr"][:, ii], OP.subtract)
            nc.scalar.activation(st["var"][:, ii], st["var"][:, ii], AF.Sqrt,
                                 bias=eps_sb[:])
            nc.vector.reciprocal_approx_fast(st["rstd"][:, ii],
                                             st["var"][:, ii])

        # attn persistents open first so everything transient frees above them
        with tc.tile_pool(name="pattn", bufs=1) as pool_attn:
            QT = pool_attn.tile([P, NQH, T], dt.bfloat16, tag="QT")
            KT = pool_attn.tile([P, NQH, T], dt.bfloat16, tag="KT")
            V = pool_attn.tile([P, NT, HH, D + 1], dt.bfloat16, tag="V")
            YT = pool_attn.tile([P, NQH, T], dt.bfloat16, tag="YT")
            wo_sb = pool_attn.tile([P, NQH, C], dt.bfloat16, tag="wo")
            nc.vector.memset(V[:, :, :, D], 1.0)

            with ExitStack() as es_zt:
                pool_zt = es_zt.enter_context(tc.tile_pool(name="pzt", bufs=1))
                ZT = pool_zt.tile([P, NC, T], dt.bfloat16)
                pool_wqkv = es_zt.enter_context(tc.tile_pool(name="pw1",
                                                             bufs=1))
                wq_sb = pool_wqkv.tile([P, NC, QH], dt.bfloat16, tag="wq")
                wk_sb = pool_wqkv.tile([P, NC, QH], dt.bfloat16, tag="wk")
                wv_sb = pool_wqkv.tile([P, NC, QH], dt.bfloat16, tag="wv")

                # ===== phase 0: stream x, LN1 stats, z, z^T =====
                with tc.tile_pool(name="pstat", bufs=1) as pool_stat, \
                     tc.tile_pool(name="pxs", bufs=2) as pool_xs, \
                     tc.tile_pool(name="ps_tra", bufs=2, space="PSUM") as ps_tra:
                    for row, full, w in ((bv_row, bv_full, QH),
                                         (bo_row, bo_full, C),
                                         (bout_row, bout_full, C)):
                        for o in range(0, w, 512):
                            wch = min(512, w - o)
                            pb = ps_tra.tile([P, 512], dt.float32, tag="bc")
                            nc.tensor.matmul(pb[:, :wch], ones1[:],
                                             row[:, o : o + wch])
                            nc.vector.tensor_copy(full[:, o : o + wch],
                                                  pb[:, :wch])

                    st1 = ln_alloc(pool_stat, NT, "ln1")
                    GRP = 4 if NT % 4 == 0 else 1
                    xg = None
                    for i in range(NT):
                        if i % GRP == 0:
                            xg = pool_xs.tile([P, GRP, C], dt.float32,
                                              tag="xg")
                            nc.sync.dma_start(
                                xg[:], x_r[:, i : i + GRP, :])
                        xc = xg[:, i % GRP, :]
                        sq = pool_xs.tile([P, C], dt.bfloat16, tag="sq")
                        nc.scalar.activation(sq[:], xc, AF.Square,
                                             accum_out=st1["s2"][:, i : i + 1])
                        ln_chunk(st1, i, xc)
                        zc = pool_xs.tile([P, C], dt.bfloat16, tag="zc")
                        nc.gpsimd.tensor_scalar(
                            zc[:], xc, st1["mean"][:, i : i + 1],
                            st1["rstd"][:, i : i + 1],
                            OP.subtract, OP.mult)
                        for jj in range(NC // NB):
                            pt = ps_tra.tile([P, NB * P], dt.bfloat16,
                                             tag="trp")
                            for j4 in range(NB):
                                j = jj * NB + j4
                                nc.tensor.transpose(
                                    pt[:, j4 * P : (j4 + 1) * P],
                                    zc[:, j * P : (j + 1) * P], id_sb[:])
                            nc.vector.tensor_copy(
                                ZT[:, jj * NB : (jj + 1) * NB,
                                   i * P : (i + 1) * P],
                                pt[:].rearrange("p (a b) -> p a b", a=NB))

                # weight DMAs issued after the x stream so the first x
                # tiles are not queued behind 5MB of weights
                nc.sync.dma_start(wq_sb[:],
                                  wq.rearrange("(ci p) o -> p ci o", p=P))
                nc.sync.dma_start(wk_sb[:],
                                  wk.rearrange("(ci p) o -> p ci o", p=P))
                nc.sync.dma_start(wv_sb[:],
                                  wv.rearrange("(ci p) o -> p ci o", p=P))
                nc.sync.dma_start(wo_sb[:],
                                  wo.rearrange("(ci p) o -> p ci o", p=P))

                # ===== fused QKV + attention + W_o + chunked RS =====
                # QKV for t-slice s is emitted just before attention q-slice
                # s; the full-width QKV matmuls are the scheduler's filler
                # for PE gaps while ScalarE works through the exps, and they
                # keep the PE activity monitor warm.
                inv_sqrt_d = 1.0 / math.sqrt(D)
                TPS = SL // P      # t-chunks per q-slice
                with tc.tile_pool(name="ppt", bufs=3) as pool_pt, \
                     tc.tile_pool(name="prec", bufs=2) as pool_rec, \
                     tc.tile_pool(name="prs", bufs=2) as pool_rs, \
                     tc.tile_pool(name="ps_pm", bufs=2, space="PSUM") as ps_pm, \
                     tc.tile_pool(name="ps_qkv", bufs=2, space="PSUM") as ps_qkv, \
                     tc.tile_pool(name="ps_av", bufs=2, space="PSUM") as ps_av, \
                     tc.tile_pool(name="ps_wo", bufs=1, space="PSUM") as ps_wo:
                    for s in range(NSL if phase_limit >= 2 else 0):
                        # --- QKV for t-slice s ---
                        for w_sb, dstT, b_sb in ((wq_sb, QT, bq_sb),
                                                 (wk_sb, KT, bk_sb)):
                            for co in range(NQH):
                                pm = ps_qkv.tile([P, TSW], dt.float32,
                                                 tag="mmp")
                                for ci in range(NC):
                                    nc.tensor.matmul(
                                        pm[:],
                                        w_sb[:, ci, co * P : (co + 1) * P],
                                        ZT[:, ci, s * TSW : (s + 1) * TSW],
                                        start=(ci == 0), stop=(ci == NC - 1))
                                nc.vector.tensor_scalar_add(
                                    dstT[:, co, s * TSW : (s + 1) * TSW],
                                    pm[:], b_sb[:, co : co + 1])
                        for ti in range(s * TPS, (s + 1) * TPS):
                            pm = ps_qkv.tile([P, TSW], dt.float32, tag="mmp")
                            for ci in range(NC):
                                nc.tensor.matmul(
                                    pm[:, :QH],
                                    ZT[:, ci, ti * P : (ti + 1) * P],
                                    wv_sb[:, ci, :],
                                    start=(ci == 0), stop=(ci == NC - 1))
                            nc.vector.tensor_tensor(
                                V[:, ti, :, :D],
                                pm[:, :QH].rearrange("p (h d) -> p h d", d=D),
                                bv_full[:].rearrange("p (h d) -> p h d", d=D),
                                OP.add)

                        # --- attention q-slice s, heads in pairs ---
                        kcm = (s + 1) * DBLK
                        for j in range(NQH):         # pair index == hc chunk
                            PT0 = pool_pt.tile([P, NT, SL], dt.bfloat16,
                                               tag="PT")
                            PT1 = pool_pt.tile([P, NT, SL], dt.bfloat16,
                                               tag="PT")
                            for kk in range(0, kcm, 2):
                                # two key chunks share a 2-bank psum tile and
                                # one exp instruction per head (the masked-off
                                # lead-in of the second band chunk is exp'd as
                                # garbage but never read downstream)
                                c0a = max(kk - s * DBLK, 0) * P
                                c0b = max(kk + 1 - s * DBLK, 0) * P
                                pm0 = ps_pm.tile([P, 2, SL], dt.float32,
                                                 tag="sp")
                                pm1 = ps_pm.tile([P, 2, SL], dt.float32,
                                                 tag="sp")
                                for dk, pmx, qt0 in ((0, pm0, 0), (0, pm1, D),
                                                     (1, pm0, 0), (1, pm1, D)):
                                    kc = kk + dk
                                    c0 = (c0a, c0b)[dk]
                                    nc.tensor.matmul(
                                        pmx[:, dk, c0:],
                                        KT[qt0 : qt0 + D, j,
                                           kc * P : (kc + 1) * P],
                                        QT[qt0 : qt0 + D, j,
                                           s * SL + c0 : (s + 1) * SL],
                                        start=True, stop=True)
                                nc.scalar.activation(
                                    PT0[:, kk : kk + 2, c0a:],
                                    pm0[:, :, c0a:], AF.Exp,
                                    scale=inv_sqrt_d)
                                nc.scalar.activation(
                                    PT1[:, kk : kk + 2, c0a:],
                                    pm1[:, :, c0a:], AF.Exp,
                                    scale=inv_sqrt_d)
                                for dk in range(2):
                                    kc = kk + dk
                                    if kc >= s * DBLK:
                                        c0 = (c0a, c0b)[dk]
                                        nc.vector.tensor_tensor(
                                            PT0[:, kc, c0 : c0 + P],
                                            PT0[:, kc, c0 : c0 + P],
                                            tri_sb[:], OP.mult)
                                        nc.vector.tensor_tensor(
                                            PT1[:, kc, c0 : c0 + P],
                                            PT1[:, kc, c0 : c0 + P],
                                            tri_sb[:], OP.mult)

                            if s >= 1:
                                for _w in range(5):
                                    dmy = ps_qkv.tile([P, TSW], dt.float32,
                                                      tag="mmp")
                                    nc.tensor.matmul(
                                        dmy[:], id_sb[:],
                                        wq_sb[:, 0, 0:TSW],
                                        start=True, stop=True)

                            for hh, PTh in ((2 * j, PT0), (2 * j + 1, PT1)):
                                hp = D * (hh % 2)
                                po = ps_av.tile([P, SL], dt.float32,
                                                tag="op")
                                for kc in range(kcm):
                                    c0 = max(kc - s * DBLK, 0) * P
                                    nc.tensor.matmul(
                                        po[: D + 1, c0:],
                                        V[:, kc, hh, :],
                                        PTh[:, kc, c0:],
                                        start=(kc == 0),
                                        stop=(kc == kcm - 1))
                                den = pool_rec.tile([1, SL], dt.float32,
                                                    tag="den")
                                recb = pool_rec.tile([D, SL], dt.float32,
                                                     tag="recb")
                                nc.vector.tensor_copy(recb[0:1, :],
                                                      po[D : D + 1, :])
                                nc.vector.reciprocal_approx_fast(
                                    den[:], recb[0:1, :])
                                nc.gpsimd.partition_broadcast(
                                    recb[:], den[:], channels=D)
                                nc.vector.tensor_tensor(
                                    YT[hp : hp + D, j,
                                       s * SL : (s + 1) * SL],
                                    po[:D, :], recb[:], OP.mult)

                        # W_o for this q-slice's t-chunks, then its RS chunk
                        for tis in range(TPS):
                            ti = s * TPS + tis
                            r_sb = pool_rs.tile([P, C], dt.bfloat16,
                                                tag="rsb")
                            for cs in range(NCS):
                                pm = ps_av.tile([P, CSW], dt.float32,
                                                tag="op")
                                for ci in range(NQH):
                                    nc.tensor.matmul(
                                        pm[:],
                                        YT[:, ci, ti * P : (ti + 1) * P],
                                        wo_sb[:, ci, cs * CSW : (cs + 1) * CSW],
                                        start=(ci == 0), stop=(ci == NQH - 1))
                                nc.vector.tensor_copy(
                                    r_sb[:, cs * CSW : (cs + 1) * CSW], pm[:])
                            nc.sync.dma_start(rb_r[:, ti, :], r_sb[:])
                        if phase_limit >= 4:
                            nc.gpsimd.collective_compute(
                                "ReduceScatter", OP.add, replica_groups=groups,
                                ins=[r_bounce[s * SL : (s + 1) * SL, :].opt()],
                                outs=[r_own_b[s * (SL // 2) :
                                              (s + 1) * (SL // 2), :].opt()])

        # ===== phase 4 + 5 =====
        with tc.tile_pool(name="px2", bufs=1) as pool_x2:
            X2 = pool_x2.tile([P, NT2, C], dt.float32, tag="x2")
            st2 = ln_alloc(pool_x2, NT2, "ln2")

            with tc.tile_pool(name="pht", bufs=1) as pool_ht:
                HT = pool_ht.tile([P, NF, T2], dt.bfloat16)

                with ExitStack() as es_z2t:
                    pool_z2t = es_z2t.enter_context(
                        tc.tile_pool(name="pz2t", bufs=1))
                    Z2Ts = [pool_z2t.tile([P, NC, TS2], dt.bfloat16,
                                          tag=f"z2t{k}", name=f"z2t_{k}")
                            for k in range(NT2S)]
                    pool_wfc = es_z2t.enter_context(
                        tc.tile_pool(name="pwfc", bufs=2))
                    ps_h = es_z2t.enter_context(
                        tc.tile_pool(name="ps_h", bufs=4, space="PSUM"))

                    # phase 4: residual + LN2 + z2 + z2^T
                    with tc.tile_pool(name="pxo", bufs=3) as pool_xo, \
                         tc.tile_pool(name="ps_trb", bufs=2,
                                      space="PSUM") as ps_trb:
                        NT2_g = NT2 if phase_limit >= 5 else 0
                        for i in range(NT2_g):
                            xoc = pool_xo.tile([P, C], dt.float32, tag="xoc")
                            roc = pool_xo.tile([P, C], dt.bfloat16, tag="roc")
                            nc.sync.dma_start(xoc[:], xo_r[:, i, :])
                            nc.sync.dma_start(roc[:], rob_r[:, i, :])
                            nc.vector.tensor_tensor(X2[:, i, :], xoc[:],
                                                    roc[:], OP.add)
                            nc.vector.tensor_tensor(
                                X2[:, i, :], X2[:, i, :], bo_full[:], OP.add)
                            sq = pool_xo.tile([P, C], dt.bfloat16, tag="sq2")
                            nc.scalar.activation(sq[:], X2[:, i, :],
                                                 AF.Square,
                                                 accum_out=st2["s2"][:, i : i + 1])
                            ln_chunk(st2, i, X2[:, i, :])
                            z2c = pool_xo.tile([P, C], dt.bfloat16, tag="z2c")
                            nc.gpsimd.tensor_scalar(
                                z2c[:], X2[:, i, :],
                                st2["mean"][:, i : i + 1],
                                st2["rstd"][:, i : i + 1],
                                OP.subtract, OP.mult)
                            for jj in range(NC // NB):
                                pt = ps_trb.tile([P, NB * P], dt.bfloat16,
                                                 tag="trp")
                                for j4 in range(NB):
                                    j = jj * NB + j4
                                    nc.tensor.transpose(
                                        pt[:, j4 * P : (j4 + 1) * P],
                                        z2c[:, j * P : (j + 1) * P], id_sb[:])
                                nc.vector.tensor_copy(
                                    Z2Ts[(i * P) // TS2][
                                        :, jj * NB : (jj + 1) * NB,
                                        (i * P) % TS2 : (i * P) % TS2 + P],
                                    pt[:].rearrange("p (a b) -> p a b", a=NB))

                    # phase 5a: FC + gelu, t-slice-major with palindrome
                    # weight streaming so the LN2 tail hides under ts_=0
                    if phase_limit >= 6:
                        for ts_ in range(NT2S):
                            fo_order = (range(FF // FCW) if ts_ % 2 == 0
                                        else reversed(range(FF // FCW)))
                            for fo in fo_order:
                                wfc_sb = pool_wfc.tile([P, NC, FCW],
                                                       dt.bfloat16, tag="wfc")
                                nc.sync.dma_start(
                                    wfc_sb[:],
                                    wfc[:, fo * FCW : (fo + 1) * FCW]
                                    .rearrange("(ci p) o -> p ci o", p=P))
                                for f in range(FCW // P):
                                    fg = fo * (FCW // P) + f
                                    pm = ps_h.tile([P, TS2], dt.float32,
                                                   tag="hp")
                                    for ci in range(NC):
                                        nc.tensor.matmul(
                                            pm[:],
                                            wfc_sb[:, ci, f * P : (f + 1) * P],
                                            Z2Ts[ts_][:, ci, :],
                                            start=(ci == 0),
                                            stop=(ci == NC - 1))
                                    nc.scalar.activation(
                                        HT[:, fg, ts_ * TS2 : (ts_ + 1) * TS2],
                                        pm[:], gelu_af,
                                        bias=bfc_sb[:, fg : fg + 1])
                es_z2t.close()

                # phase 5b: W_out + residual
                with tc.tile_pool(name="pwout", bufs=3) as pool_wout, \
                     tc.tile_pool(name="pout", bufs=3) as pool_out, \
                     tc.tile_pool(name="ps_out", bufs=1,
                                  space="PSUM") as ps_out:
                    for cs in range(NCS if phase_limit >= 7 else 0):
                        pms = [ps_out.tile([P, CSW], dt.float32,
                                           tag=f"outp{ti}",
                                           name=f"outp_{cs}_{ti}")
                               for ti in range(NT2)]
                        for fi in range(NF):
                            wout_sb = pool_wout.tile([P, CSW], dt.bfloat16,
                                                     tag="wout")
                            nc.sync.dma_start(
                                wout_sb[:],
                                wout[fi * P : (fi + 1) * P,
                                     cs * CSW : (cs + 1) * CSW])
                            for ti in range(NT2):
                                nc.tensor.matmul(
                                    pms[ti][:],
                                    HT[:, fi, ti * P : (ti + 1) * P],
                                    wout_sb[:],
                                    start=(fi == 0), stop=(fi == NF - 1))
                        for ti in range(NT2):
                            o_sb = pool_out.tile([P, CSW], dt.float32,
                                                 tag="osb")
                            nc.vector.tensor_tensor(
                                o_sb[:], pms[ti][:],
                                X2[:, ti, cs * CSW : (cs + 1) * CSW], OP.add)
                            nc.vector.tensor_tensor(
                                o_sb[:], o_sb[:],
                                bout_full[:, cs * CSW : (cs + 1) * CSW],
                                OP.add)
                            nc.sync.dma_start(
                                out_r[:, ti, cs * CSW : (cs + 1) * CSW],
                                o_sb[:])

    nc.compile()
    return nc


def _prep_core_inputs(b, parity, x, ln1_w, ln1_b, w_qkv, b_qkv, w_o, b_o,
                      ln2_w, ln2_b, w_fc, b_fc, w_out, b_out,
                      T_, C_, H_, D_):
    """Host-side per-core input dict (weights LN-folded, matmul inputs bf16)."""
    bf16 = ml_dtypes.bfloat16
    HH = H_ // 2
    QH = HH * D_
    T2 = T_ // 2
    wq_eff = (ln1_w[:, None] * w_qkv).astype(np.float32)
    bq_eff = (b_qkv + ln1_b @ w_qkv).astype(np.float32)
    wfc_eff = (ln2_w[:, None] * w_fc).astype(np.float32)
    bfc_eff = (b_fc + ln2_b @ w_fc).astype(np.float32)

    h0 = parity * QH
    sl_q = slice(h0, h0 + QH)
    sl_k = slice(C_ + h0, C_ + h0 + QH)
    sl_v = slice(2 * C_ + h0, 2 * C_ + h0 + QH)
    tri = np.tril(np.ones((P, P), np.float32)).T  # tri[k,q] = 1 if k <= q
    ident = np.eye(P, dtype=np.float32)
    SL_ = min(512, T_)
    HS = SL_ // 2
    own_rows = np.concatenate([
        np.arange(s * SL_ + parity * HS, s * SL_ + (parity + 1) * HS)
        for s in range(T_ // SL_)])
    return {
        "x_full": np.ascontiguousarray(x[b]),
        "x_own": np.ascontiguousarray(x[b, own_rows]),
        "wq": np.ascontiguousarray(wq_eff[:, sl_q]).astype(bf16),
        "wk": np.ascontiguousarray(wq_eff[:, sl_k]).astype(bf16),
        "wv": np.ascontiguousarray(wq_eff[:, sl_v]).astype(bf16),
        "bq": np.ascontiguousarray(bq_eff[sl_q]),
        "bk": np.ascontiguousarray(bq_eff[sl_k]),
        "bv": np.ascontiguousarray(bq_eff[sl_v]),
        "wo": np.ascontiguousarray(w_o[h0 : h0 + QH, :]).astype(bf16),
        "bo": np.ascontiguousarray(b_o),
        "wfc": np.ascontiguousarray(wfc_eff).astype(bf16),
        "bfc": np.ascontiguousarray(bfc_eff),
        "wout": np.ascontiguousarray(w_out).astype(bf16),
        "bout": np.ascontiguousarray(b_out),
        "tri": tri.astype(bf16),
        "ident": ident.astype(bf16),
    }


def kernel(x, ln1_w, ln1_b, w_qkv, b_qkv, w_o, b_o, ln2_w, ln2_b,
           w_fc, b_fc, w_out, b_out):
    from concourse.bass_utils import run_bass_kernel_spmd

    key = (T, C, H, D, FF, N_CORES)
    if key not in _CACHE:
        groups = [[2 * i, 2 * i + 1] for i in range(N_CORES // 2)]
        _CACHE[key] = _build(T, C, H, D, FF, N_CORES, groups)
    nc = _CACHE[key]

    args = (np.asarray(x, np.float32), np.asarray(ln1_w, np.float32),
            np.asarray(ln1_b, np.float32), np.asarray(w_qkv, np.float32),
            np.asarray(b_qkv, np.float32), np.asarray(w_o, np.float32),
            np.asarray(b_o, np.float32), np.asarray(ln2_w, np.float32),
            np.asarray(ln2_b, np.float32), np.asarray(w_fc, np.float32),
            np.asarray(b_fc, np.float32), np.asarray(w_out, np.float32),
            np.asarray(b_out, np.float32))
    in_maps = []
    for core in range(N_CORES):
        b, parity = core // 2, core % 2
        in_maps.append(_prep_core_inputs(b, parity, *args, T, C, H, D))

    global LAST_RESULT
    res = run_bass_kernel_spmd(nc, in_maps, core_ids=list(range(N_CORES)))
    LAST_RESULT = res

    SL_ = min(512, T)
    HS = SL_ // 2
    full = np.empty((B, T, C), np.float32)
    for core in range(N_CORES):
        b, parity = core // 2, core % 2
        o = res.results[core]["out"]
        for s in range(T // SL_):
            full[b, s * SL_ + parity * HS : s * SL_ + (parity + 1) * HS] = \
                o[s * HS : (s + 1) * HS]
    return full


# revision 19
# speedup vs baseline: 1.4455x; 1.4455x over previous
"""Trainium2 Bass kernel for a dense transformer block (B=4,T=2048,H=16,D=64,C=1024,FF=4096).

Sharding: batch b -> core pair (2b, 2b+1). Within a pair, attention is split by
heads (8 heads/core, Megatron column-parallel QKV + row-parallel W_o), the
attention output partial sums are combined with a pair ReduceScatter, and each
core then runs the full-FF MLP on its half (1024) of the rows.

v2 attention pipeline (per 512-wide q-slice, heads processed in groups of 4):
  - S^T = K @ Q^T per head pair with 64-row PE tiling: the two heads of a pair
    live at SBUF partitions 0-63 / 64-127, so their K=64 matmuls go to PE row
    tiles (0,0)/(64,0) and run concurrently.
  - exp on ScalarE (the attention bottleneck engine) -> PT bf16.
  - softmax denominators via M=1 matmuls (ones_col.T @ PT) 4-way column-tiled
    to PSUM partitions 0/32/64/96 of one bank, accumulated over key chunks.
  - One batched reciprocal per 4-head group, recip broadcast to 64 partitions
    via paired fp32 matmuls (row tiles 0/32/64/96 x col tiles 0/64).
  - AV as V^T @ PT with M=64 column-tiled pairs: head A -> PSUM 0-63, head B
    -> 64-127 of one bank; denominator folded in afterwards by one DVE
    multiply per pair (po * recb) straight into YT.
  - Fully-masked key blocks are never computed; the partially-masked diagonal
    128-block is exp'd then multiplied by a constant triangle (DVE). The
    region left of the diagonal is skipped via subrange accumulation in the
    den/AV matmuls (kc=0 always starts full-width), so no memsets.

LayerNorm affines are folded into the following matmul weights on the host.
Matmuls run in bf16 with fp32 PSUM accumulation. QKV is produced per-512-slice
(Q then K then V) so attention starts while QKV is still streaming; QKV bias
adds run on VectorE to keep ScalarE free for exp. The MLP FC loop runs
t-slice-major (palindrome over weight chunks) so the last ReduceScatter chunk
and LN2 tail hide under the first FC t-slice.
"""

import math

import ml_dtypes
import numpy as np

P = 128
B, T, H, D = 4, 2048, 16, 64
C = H * D
FF = 4096
EPS = 1e-5
N_CORES = 8

_CACHE = {}
LAST_RESULT = None


def _build(T, C, H, D, FF, n_cores, groups, phase_limit=99, sim_safe=False):
    """Build + compile the single-core SPMD program. Returns the Bacc object."""
    from contextlib import ExitStack

    import concourse.mybir as mybir
    import concourse.tile as tile
    from concourse import bacc

    dt = mybir.dt
    AF = mybir.ActivationFunctionType
    OP = mybir.AluOpType

    HH = H // 2               # heads per core
    QH = HH * D               # per-core c_out for each of q,k,v
    NQH = QH // P
    NT = T // P
    T2 = T // 2               # own rows
    NT2 = T2 // P
    NC = C // P
    NF = FF // P
    SL = min(512, T)          # attention q-slice width
    NSL = T // SL
    DBLK = SL // P
    HPC = P // D              # heads per 128-partition chunk (=2)
    FCW = min(512, FF)        # wfc col-chunk width
    TSW = min(512, T)         # qkv t-slice width
    NTS = T // TSW
    CSW = min(512, C)
    NCS = C // CSW
    TS2 = min(512, T2)
    NT2S = T2 // TS2
    NB = 4 if NC % 4 == 0 else 1  # transposes batched per psum bank
    assert QH % P == 0 and T % SL == 0 and SL % P == 0

    nc = bacc.Bacc("TRN2", target_bir_lowering=False, debug=False,
                   num_devices=n_cores)
    gelu_af = (mybir.ActivationFunctionType.Identity if sim_safe
               else mybir.ActivationFunctionType.Gelu)

    # ---- kernel I/O ----
    x_full = nc.dram_tensor("x_full", [T, C], dt.float32, kind="ExternalInput")
    x_own = nc.dram_tensor("x_own", [T2, C], dt.float32, kind="ExternalInput")
    wq = nc.dram_tensor("wq", [C, QH], dt.bfloat16, kind="ExternalInput")
    wk = nc.dram_tensor("wk", [C, QH], dt.bfloat16, kind="ExternalInput")
    wv = nc.dram_tensor("wv", [C, QH], dt.bfloat16, kind="ExternalInput")
    bq = nc.dram_tensor("bq", [QH], dt.float32, kind="ExternalInput")
    bk = nc.dram_tensor("bk", [QH], dt.float32, kind="ExternalInput")
    bv = nc.dram_tensor("bv", [QH], dt.float32, kind="ExternalInput")
    wo = nc.dram_tensor("wo", [QH, C], dt.bfloat16, kind="ExternalInput")
    bo = nc.dram_tensor("bo", [C], dt.float32, kind="ExternalInput")
    wfc = nc.dram_tensor("wfc", [C, FF], dt.bfloat16, kind="ExternalInput")
    bfc = nc.dram_tensor("bfc", [FF], dt.float32, kind="ExternalInput")
    wout = nc.dram_tensor("wout", [FF, C], dt.bfloat16, kind="ExternalInput")
    bout = nc.dram_tensor("bout", [C], dt.float32, kind="ExternalInput")
    tri = nc.dram_tensor("tri", [P, P], dt.bfloat16, kind="ExternalInput")
    ident = nc.dram_tensor("ident", [P, P], dt.bfloat16, kind="ExternalInput")
    out = nc.dram_tensor("out", [T2, C], dt.float32, kind="ExternalOutput")

    # collective bounce buffers (internal DRAM)
    r_bounce = nc.dram_tensor("r_bounce", [T, C], dt.bfloat16)
    r_own_b = nc.dram_tensor("r_own_b", [T2, C], dt.bfloat16)

    x_r = x_full.rearrange("(i p) c -> p i c", p=P)
    xo_r = x_own.rearrange("(i p) c -> p i c", p=P)
    out_r = out.rearrange("(i p) c -> p i c", p=P)
    rb_r = r_bounce.rearrange("(i p) c -> p i c", p=P)
    rob_r = r_own_b.rearrange("(i p) c -> p i c", p=P)

    with tile.TileContext(nc) as tc, ExitStack() as stk:
        pool_const = stk.enter_context(tc.tile_pool(name="const", bufs=1))

        tri_sb = pool_const.tile([P, P], dt.bfloat16)
        id_sb = pool_const.tile([P, P], dt.bfloat16)
        nc.sync.dma_start(tri_sb[:], tri[:])
        nc.sync.dma_start(id_sb[:], ident[:])
        bq_sb = pool_const.tile([P, NQH], dt.float32)
        bk_sb = pool_const.tile([P, NQH], dt.float32)
        bv_row = pool_const.tile([1, QH], dt.float32)
        bo_row = pool_const.tile([1, C], dt.float32)
        bfc_sb = pool_const.tile([P, NF], dt.float32)
        bout_row = pool_const.tile([1, C], dt.float32)
        eps_sb = pool_const.tile([P, 1], dt.float32)
        nc.vector.memset(eps_sb[:], EPS)
        ones1 = pool_const.tile([1, P], dt.float32)
        nc.vector.memset(ones1[:], 1.0)
        bv_full = pool_const.tile([P, QH], dt.bfloat16)
        bo_full = pool_const.tile([P, C], dt.bfloat16)
        bout_full = pool_const.tile([P, C], dt.bfloat16)
        nc.sync.dma_start(bq_sb[:], bq.rearrange("(a p) -> p a", p=P))
        nc.sync.dma_start(bk_sb[:], bk.rearrange("(a p) -> p a", p=P))
        nc.sync.dma_start(bv_row[:], bv[None, :])
        nc.sync.dma_start(bo_row[:], bo[None, :])
        nc.sync.dma_start(bfc_sb[:], bfc.rearrange("(a p) -> p a", p=P))
        nc.sync.dma_start(bout_row[:], bout[None, :])

        def ln_alloc(pool, n_chunks, nm):
            tiles = {}
            for t in ("s1", "s2", "mean", "var", "rstd", "nmr"):
                tiles[t] = pool.tile([P, n_chunks], dt.float32,
                                     tag=f"ln_{t}", name=f"{t}_{nm}")
            return tiles

        def ln_chunk(st, i, xc):
            # stats + rstd/-mean*rstd for one [P, C] chunk (column i)
            ii = slice(i, i + 1)
            nc.vector.reduce_sum(st["s1"][:, ii], xc,
                                 axis=mybir.AxisListType.X)
            nc.vector.tensor_scalar_mul(st["mean"][:, ii], st["s1"][:, ii],
                                        1.0 / C)
            nc.vector.tensor_scalar_mul(st["var"][:, ii], st["s2"][:, ii],
                                        1.0 / C)
            nc.vector.tensor_tensor(st["nmr"][:, ii], st["mean"][:, ii],
                                    st["mean"][:, ii], OP.mult)
            nc.vector.tensor_tensor(st["var"][:, ii], st["var"][:, ii],
                                    st["nmr"][:, ii], OP.subtract)
            nc.scalar.activation(st["var"][:, ii], st["var"][:, ii], AF.Sqrt,
                                 bias=eps_sb[:])
            nc.vector.reciprocal_approx_fast(st["rstd"][:, ii],
                                             st["var"][:, ii])
            nc.vector.tensor_tensor(st["nmr"][:, ii], st["mean"][:, ii],
                                    st["rstd"][:, ii], OP.mult)
            nc.vector.tensor_scalar_mul(st["nmr"][:, ii], st["nmr"][:, ii],
                                        -1.0)

        # attn persistents open first so everything transient frees above them
        with tc.tile_pool(name="pattn", bufs=1) as pool_attn:
            QT = pool_attn.tile([P, NQH, T], dt.bfloat16, tag="QT")
            KT = pool_attn.tile([P, NQH, T], dt.bfloat16, tag="KT")
            V = pool_attn.tile([P, NT, HH, D + 1], dt.bfloat16, tag="V")
            YT = pool_attn.tile([P, NQH, T], dt.bfloat16, tag="YT")
            wo_sb = pool_attn.tile([P, NQH, C], dt.bfloat16, tag="wo")
            nc.vector.memset(V[:, :, :, D], 1.0)

            with ExitStack() as es_zt:
                pool_zt = es_zt.enter_context(tc.tile_pool(name="pzt", bufs=1))
                ZT = pool_zt.tile([P, NC, T], dt.bfloat16)
                pool_wqkv = es_zt.enter_context(tc.tile_pool(name="pw1",
                                                             bufs=1))
                wq_sb = pool_wqkv.tile([P, NC, QH], dt.bfloat16, tag="wq")
                wk_sb = pool_wqkv.tile([P, NC, QH], dt.bfloat16, tag="wk")
                wv_sb = pool_wqkv.tile([P, NC, QH], dt.bfloat16, tag="wv")

                # ===== phase 0: stream x, LN1 stats, z, z^T =====
                with tc.tile_pool(name="pstat", bufs=1) as pool_stat, \
                     tc.tile_pool(name="pxs", bufs=2) as pool_xs, \
                     tc.tile_pool(name="ps_tra", bufs=2, space="PSUM") as ps_tra:
                    for row, full, w in ((bv_row, bv_full, QH),
                                         (bo_row, bo_full, C),
                                         (bout_row, bout_full, C)):
                        for o in range(0, w, 512):
                            wch = min(512, w - o)
                            pb = ps_tra.tile([P, 512], dt.float32, tag="bc")
                            nc.tensor.matmul(pb[:, :wch], ones1[:],
                                             row[:, o : o + wch])
                            nc.vector.tensor_copy(full[:, o : o + wch],
                                                  pb[:, :wch])

                    st1 = ln_alloc(pool_stat, NT, "ln1")
                    GRP = 4 if NT % 4 == 0 else 1
                    xg = None
                    for i in range(NT):
                        if i % GRP == 0:
                            xg = pool_xs.tile([P, GRP, C], dt.float32,
                                              tag="xg")
                            nc.sync.dma_start(
                                xg[:], x_r[:, i : i + GRP, :])
                        xc = xg[:, i % GRP, :]
                        sq = pool_xs.tile([P, C], dt.bfloat16, tag="sq")
                        nc.scalar.activation(sq[:], xc, AF.Square,
                                             accum_out=st1["s2"][:, i : i + 1])
                        ln_chunk(st1, i, xc)
                        zc = pool_xs.tile([P, C], dt.bfloat16, tag="zc")
                        nc.scalar.activation(zc[:], xc, AF.Identity,
                                             bias=st1["nmr"][:, i : i + 1],
                                             scale=st1["rstd"][:, i : i + 1])
                        for jj in range(NC // NB):
                            pt = ps_tra.tile([P, NB * P], dt.bfloat16,
                                             tag="trp")
                            for j4 in range(NB):
                                j = jj * NB + j4
                                nc.tensor.transpose(
                                    pt[:, j4 * P : (j4 + 1) * P],
                                    zc[:, j * P : (j + 1) * P], id_sb[:])
                            nc.vector.tensor_copy(
                                ZT[:, jj * NB : (jj + 1) * NB,
                                   i * P : (i + 1) * P],
                                pt[:].rearrange("p (a b) -> p a b", a=NB))

                # weight DMAs issued after the x stream so the first x
                # tiles are not queued behind 5MB of weights
                nc.sync.dma_start(wq_sb[:],
                                  wq.rearrange("(ci p) o -> p ci o", p=P))
                nc.sync.dma_start(wk_sb[:],
                                  wk.rearrange("(ci p) o -> p ci o", p=P))
                nc.sync.dma_start(wv_sb[:],
                                  wv.rearrange("(ci p) o -> p ci o", p=P))
                nc.sync.dma_start(wo_sb[:],
                                  wo.rearrange("(ci p) o -> p ci o", p=P))

                # ===== fused QKV + attention + W_o + chunked RS =====
                # QKV for t-slice s is emitted just before attention q-slice
                # s; the full-width QKV matmuls are the scheduler's filler
                # for PE gaps while ScalarE works through the exps, and they
                # keep the PE activity monitor warm.
                inv_sqrt_d = 1.0 / math.sqrt(D)
                TPS = SL // P      # t-chunks per q-slice
                with tc.tile_pool(name="ppt", bufs=3) as pool_pt, \
                     tc.tile_pool(name="prec", bufs=2) as pool_rec, \
                     tc.tile_pool(name="prs", bufs=2) as pool_rs, \
                     tc.tile_pool(name="ps_pm", bufs=2, space="PSUM") as ps_pm, \
                     tc.tile_pool(name="ps_qkv", bufs=2, space="PSUM") as ps_qkv, \
                     tc.tile_pool(name="ps_av", bufs=2, space="PSUM") as ps_av, \
                     tc.tile_pool(name="ps_wo", bufs=1, space="PSUM") as ps_wo:
                    for s in range(NSL if phase_limit >= 2 else 0):
                        # --- QKV for t-slice s ---
                        for w_sb, dstT, b_sb in ((wq_sb, QT, bq_sb),
                                                 (wk_sb, KT, bk_sb)):
                            for co in range(NQH):
                                pm = ps_qkv.tile([P, TSW], dt.float32,
                                                 tag="mmp")
                                for ci in range(NC):
                                    nc.tensor.matmul(
                                        pm[:],
                                        w_sb[:, ci, co * P : (co + 1) * P],
                                        ZT[:, ci, s * TSW : (s + 1) * TSW],
                                        start=(ci == 0), stop=(ci == NC - 1))
                                nc.vector.tensor_scalar_add(
                                    dstT[:, co, s * TSW : (s + 1) * TSW],
                                    pm[:], b_sb[:, co : co + 1])
                        for ti in range(s * TPS, (s + 1) * TPS):
                            pm = ps_qkv.tile([P, TSW], dt.float32, tag="mmp")
                            for ci in range(NC):
                                nc.tensor.matmul(
                                    pm[:, :QH],
                                    ZT[:, ci, ti * P : (ti + 1) * P],
                                    wv_sb[:, ci, :],
                                    start=(ci == 0), stop=(ci == NC - 1))
                            nc.vector.tensor_tensor(
                                V[:, ti, :, :D],
                                pm[:, :QH].rearrange("p (h d) -> p h d", d=D),
                                bv_full[:].rearrange("p (h d) -> p h d", d=D),
                                OP.add)

                        # --- attention q-slice s, heads in pairs ---
                        kcm = (s + 1) * DBLK
                        for j in range(NQH):         # pair index == hc chunk
                            PT0 = pool_pt.tile([P, NT, SL], dt.bfloat16,
                                               tag="PT")
                            PT1 = pool_pt.tile([P, NT, SL], dt.bfloat16,
                                               tag="PT")
                            for kk in range(0, kcm, 2):
                                # two key chunks share a 2-bank psum tile and
                                # one exp instruction per head (the masked-off
                                # lead-in of the second band chunk is exp'd as
                                # garbage but never read downstream)
                                c0a = max(kk - s * DBLK, 0) * P
                                c0b = max(kk + 1 - s * DBLK, 0) * P
                                pm0 = ps_pm.tile([P, 2, SL], dt.float32,
                                                 tag="sp")
                                pm1 = ps_pm.tile([P, 2, SL], dt.float32,
                                                 tag="sp")
                                for dk, pmx, qt0 in ((0, pm0, 0), (0, pm1, D),
                                                     (1, pm0, 0), (1, pm1, D)):
                                    kc = kk + dk
                                    c0 = (c0a, c0b)[dk]
                                    nc.tensor.matmul(
                                        pmx[:, dk, c0:],
                                        KT[qt0 : qt0 + D, j,
                                           kc * P : (kc + 1) * P],
                                        QT[qt0 : qt0 + D, j,
                                           s * SL + c0 : (s + 1) * SL],
                                        start=True, stop=True)
                                nc.scalar.activation(
                                    PT0[:, kk : kk + 2, c0a:],
                                    pm0[:, :, c0a:], AF.Exp,
                                    scale=inv_sqrt_d)
                                nc.scalar.activation(
                                    PT1[:, kk : kk + 2, c0a:],
                                    pm1[:, :, c0a:], AF.Exp,
                                    scale=inv_sqrt_d)
                                for dk in range(2):
                                    kc = kk + dk
                                    if kc >= s * DBLK:
                                        c0 = (c0a, c0b)[dk]
                                        nc.vector.tensor_tensor(
                                            PT0[:, kc, c0 : c0 + P],
                                            PT0[:, kc, c0 : c0 + P],
                                            tri_sb[:], OP.mult)
                                        nc.vector.tensor_tensor(
                                            PT1[:, kc, c0 : c0 + P],
                                            PT1[:, kc, c0 : c0 + P],
                                            tri_sb[:], OP.mult)

                            if s >= 1:
                                for _w in range(5):
                                    dmy = ps_qkv.tile([P, TSW], dt.float32,
                                                      tag="mmp")
                                    nc.tensor.matmul(
                                        dmy[:], id_sb[:],
                                        wq_sb[:, 0, 0:TSW],
                                        start=True, stop=True)

                            for hh, PTh in ((2 * j, PT0), (2 * j + 1, PT1)):
                                hp = D * (hh % 2)
                                po = ps_av.tile([P, SL], dt.float32,
                                                tag="op")
                                for kc in range(kcm):
                                    c0 = max(kc - s * DBLK, 0) * P
                                    nc.tensor.matmul(
                                        po[: D + 1, c0:],
                                        V[:, kc, hh, :],
                                        PTh[:, kc, c0:],
                                        start=(kc == 0),
                                        stop=(kc == kcm - 1))
                                den = pool_rec.tile([1, SL], dt.float32,
                                                    tag="den")
                                recb = pool_rec.tile([D, SL], dt.float32,
                                                     tag="recb")
                                nc.vector.tensor_copy(recb[0:1, :],
                                                      po[D : D + 1, :])
                                nc.vector.reciprocal_approx_fast(
                                    den[:], recb[0:1, :])
                                nc.gpsimd.partition_broadcast(
                                    recb[:], den[:], channels=D)
                                nc.vector.tensor_tensor(
                                    YT[hp : hp + D, j,
                                       s * SL : (s + 1) * SL],
                                    po[:D, :], recb[:], OP.mult)

                        # W_o for this q-slice's t-chunks, then its RS chunk
                        for tis in range(TPS):
                            ti = s * TPS + tis
                            r_sb = pool_rs.tile([P, C], dt.bfloat16,
                                                tag="rsb")
                            for cs in range(NCS):
                                pm = ps_av.tile([P, CSW], dt.float32,
                                                tag="op")
                                for ci in range(NQH):
                                    nc.tensor.matmul(
                                        pm[:],
                                        YT[:, ci, ti * P : (ti + 1) * P],
                                        wo_sb[:, ci, cs * CSW : (cs + 1) * CSW],
                                        start=(ci == 0), stop=(ci == NQH - 1))
                                nc.vector.tensor_copy(
                                    r_sb[:, cs * CSW : (cs + 1) * CSW], pm[:])
                            nc.sync.dma_start(rb_r[:, ti, :], r_sb[:])
                        if phase_limit >= 4:
                            nc.gpsimd.collective_compute(
                                "ReduceScatter", OP.add, replica_groups=groups,
                                ins=[r_bounce[s * SL : (s + 1) * SL, :].opt()],
                                outs=[r_own_b[s * (SL // 2) :
                                              (s + 1) * (SL // 2), :].opt()])

        # ===== phase 4 + 5 =====
        with tc.tile_pool(name="px2", bufs=1) as pool_x2:
            X2 = pool_x2.tile([P, NT2, C], dt.float32, tag="x2")
            st2 = ln_alloc(pool_x2, NT2, "ln2")

            with tc.tile_pool(name="pht", bufs=1) as pool_ht:
                HT = pool_ht.tile([P, NF, T2], dt.bfloat16)

                with ExitStack() as es_z2t:
                    pool_z2t = es_z2t.enter_context(
                        tc.tile_pool(name="pz2t", bufs=1))
                    Z2Ts = [pool_z2t.tile([P, NC, TS2], dt.bfloat16,
                                          tag=f"z2t{k}", name=f"z2t_{k}")
                            for k in range(NT2S)]
                    pool_wfc = es_z2t.enter_context(
                        tc.tile_pool(name="pwfc", bufs=2))
                    ps_h = es_z2t.enter_context(
                        tc.tile_pool(name="ps_h", bufs=4, space="PSUM"))

                    # phase 4: residual + LN2 + z2 + z2^T
                    with tc.tile_pool(name="pxo", bufs=3) as pool_xo, \
                         tc.tile_pool(name="ps_trb", bufs=2,
                                      space="PSUM") as ps_trb:
                        NT2_g = NT2 if phase_limit >= 5 else 0
                        for i in range(NT2_g):
                            xoc = pool_xo.tile([P, C], dt.float32, tag="xoc")
                            roc = pool_xo.tile([P, C], dt.bfloat16, tag="roc")
                            nc.sync.dma_start(xoc[:], xo_r[:, i, :])
                            nc.sync.dma_start(roc[:], rob_r[:, i, :])
                            nc.vector.tensor_tensor(X2[:, i, :], xoc[:],
                                                    roc[:], OP.add)
                            nc.vector.tensor_tensor(
                                X2[:, i, :], X2[:, i, :], bo_full[:], OP.add)
                            sq = pool_xo.tile([P, C], dt.bfloat16, tag="sq2")
                            nc.scalar.activation(sq[:], X2[:, i, :],
                                                 AF.Square,
                                                 accum_out=st2["s2"][:, i : i + 1])
                            ln_chunk(st2, i, X2[:, i, :])
                            z2c = pool_xo.tile([P, C], dt.bfloat16, tag="z2c")
                            nc.scalar.activation(z2c[:], X2[:, i, :],
                                                 AF.Identity,
                                                 bias=st2["nmr"][:, i : i + 1],
                                                 scale=st2["rstd"][:, i : i + 1])
                            for jj in range(NC // NB):
                                pt = ps_trb.tile([P, NB * P], dt.bfloat16,
                                                 tag="trp")
                                for j4 in range(NB):
                                    j = jj * NB + j4
                                    nc.tensor.transpose(
                                        pt[:, j4 * P : (j4 + 1) * P],
                                        z2c[:, j * P : (j + 1) * P], id_sb[:])
                                nc.vector.tensor_copy(
                                    Z2Ts[(i * P) // TS2][
                                        :, jj * NB : (jj + 1) * NB,
                                        (i * P) % TS2 : (i * P) % TS2 + P],
                                    pt[:].rearrange("p (a b) -> p a b", a=NB))

                    # phase 5a: FC + gelu, t-slice-major with palindrome
                    # weight streaming so the LN2 tail hides under ts_=0
                    if phase_limit >= 6:
                        for ts_ in range(NT2S):
                            fo_order = (range(FF // FCW) if ts_ % 2 == 0
                                        else reversed(range(FF // FCW)))
                            for fo in fo_order:
                                wfc_sb = pool_wfc.tile([P, NC, FCW],
                                                       dt.bfloat16, tag="wfc")
                                nc.sync.dma_start(
                                    wfc_sb[:],
                                    wfc[:, fo * FCW : (fo + 1) * FCW]
                                    .rearrange("(ci p) o -> p ci o", p=P))
                                for f in range(FCW // P):
                                    fg = fo * (FCW // P) + f
                                    pm = ps_h.tile([P, TS2], dt.float32,
                                                   tag="hp")
                                    for ci in range(NC):
                                        nc.tensor.matmul(
                                            pm[:],
                                            wfc_sb[:, ci, f * P : (f + 1) * P],
                                            Z2Ts[ts_][:, ci, :],
                                            start=(ci == 0),
                                            stop=(ci == NC - 1))
                                    nc.scalar.activation(
                                        HT[:, fg, ts_ * TS2 : (ts_ + 1) * TS2],
                                        pm[:], gelu_af,
                                        bias=bfc_sb[:, fg : fg + 1])
                es_z2t.close()

                # phase 5b: W_out + residual
                with tc.tile_pool(name="pwout", bufs=3) as pool_wout, \
                     tc.tile_pool(name="pout", bufs=3) as pool_out, \
                     tc.tile_pool(name="ps_out", bufs=1,
                                  space="PSUM") as ps_out:
                    for cs in range(NCS if phase_limit >= 7 else 0):
                        pms = [ps_out.tile([P, CSW], dt.float32,
                                           tag=f"outp{ti}",
                                           name=f"outp_{cs}_{ti}")
                               for ti in range(NT2)]
                        for fi in range(NF):
                            wout_sb = pool_wout.tile([P, CSW], dt.bfloat16,
                                                     tag="wout")
                            nc.sync.dma_start(
                                wout_sb[:],
                                wout[fi * P : (fi + 1) * P,
                                     cs * CSW : (cs + 1) * CSW])
                            for ti in range(NT2):
                                nc.tensor.matmul(
                                    pms[ti][:],
                                    HT[:, fi, ti * P : (ti + 1) * P],
                                    wout_sb[:],
                                    start=(fi == 0), stop=(fi == NF - 1))
                        for ti in range(NT2):
                            o_sb = pool_out.tile([P, CSW], dt.float32,
                                                 tag="osb")
                            nc.vector.tensor_tensor(
                                o_sb[:], pms[ti][:],
                                X2[:, ti, cs * CSW : (cs + 1) * CSW], OP.add)
                            nc.vector.tensor_tensor(
                                o_sb[:], o_sb[:],
                                bout_full[:, cs * CSW : (cs + 1) * CSW],
                                OP.add)
                            nc.sync.dma_start(
                                out_r[:, ti, cs * CSW : (cs + 1) * CSW],
                                o_sb[:])

    nc.compile()
    return nc


def _prep_core_inputs(b, parity, x, ln1_w, ln1_b, w_qkv, b_qkv, w_o, b_o,
                      ln2_w, ln2_b, w_fc, b_fc, w_out, b_out,
                      T_, C_, H_, D_):
    """Host-side per-core input dict (weights LN-folded, matmul inputs bf16)."""
    bf16 = ml_dtypes.bfloat16
    HH = H_ // 2
    QH = HH * D_
    T2 = T_ // 2
    wq_eff = (ln1_w[:, None] * w_qkv).astype(np.float32)
    bq_eff = (b_qkv + ln1_b @ w_qkv).astype(np.float32)
    wfc_eff = (ln2_w[:, None] * w_fc).astype(np.float32)
    bfc_eff = (b_fc + ln2_b @ w_fc).astype(np.float32)

    h0 = parity * QH
    sl_q = slice(h0, h0 + QH)
    sl_k = slice(C_ + h0, C_ + h0 + QH)
    sl_v = slice(2 * C_ + h0, 2 * C_ + h0 + QH)
    tri = np.tril(np.ones((P, P), np.float32)).T  # tri[k,q] = 1 if k <= q
    ident = np.eye(P, dtype=np.float32)
    SL_ = min(512, T_)
    HS = SL_ // 2
    own_rows = np.concatenate([
        np.arange(s * SL_ + parity * HS, s * SL_ + (parity + 1) * HS)
        for s in range(T_ // SL_)])
    return {
        "x_full": np.ascontiguousarray(x[b]),
        "x_own": np.ascontiguousarray(x[b, own_rows]),
        "wq": np.ascontiguousarray(wq_eff[:, sl_q]).astype(bf16),
        "wk": np.ascontiguousarray(wq_eff[:, sl_k]).astype(bf16),
        "wv": np.ascontiguousarray(wq_eff[:, sl_v]).astype(bf16),
        "bq": np.ascontiguousarray(bq_eff[sl_q]),
        "bk": np.ascontiguousarray(bq_eff[sl_k]),
        "bv": np.ascontiguousarray(bq_eff[sl_v]),
        "wo": np.ascontiguousarray(w_o[h0 : h0 + QH, :]).astype(bf16),
        "bo": np.ascontiguousarray(b_o),
        "wfc": np.ascontiguousarray(wfc_eff).astype(bf16),
        "bfc": np.ascontiguousarray(bfc_eff),
        "wout": np.ascontiguousarray(w_out).astype(bf16),
        "bout": np.ascontiguousarray(b_out),
        "tri": tri.astype(bf16),
        "ident": ident.astype(bf16),
    }


def kernel(x, ln1_w, ln1_b, w_qkv, b_qkv, w_o, b_o, ln2_w, ln2_b,
           w_fc, b_fc, w_out, b_out):
    from concourse.bass_utils import run_bass_kernel_spmd

    key = (T, C, H, D, FF, N_CORES)
    if key not in _CACHE:
        groups = [[2 * i, 2 * i + 1] for i in range(N_CORES // 2)]
        _CACHE[key] = _build(T, C, H, D, FF, N_CORES, groups)
    nc = _CACHE[key]

    args = (np.asarray(x, np.float32), np.asarray(ln1_w, np.float32),
            np.asarray(ln1_b, np.float32), np.asarray(w_qkv, np.float32),
            np.asarray(b_qkv, np.float32), np.asarray(w_o, np.float32),
            np.asarray(b_o, np.float32), np.asarray(ln2_w, np.float32),
            np.asarray(ln2_b, np.float32), np.asarray(w_fc, np.float32),
            np.asarray(b_fc, np.float32), np.asarray(w_out, np.float32),
            np.asarray(b_out, np.float32))
    in_maps = []
    for core in range(N_CORES):
        b, parity = core // 2, core % 2
        in_maps.append(_prep_core_inputs(b, parity, *args, T, C, H, D))

    global LAST_RESULT
    res = run_bass_kernel_spmd(nc, in_maps, core_ids=list(range(N_CORES)))
    LAST_RESULT = res

    SL_ = min(512, T)
    HS = SL_ // 2
    full = np.empty((B, T, C), np.float32)
    for core in range(N_CORES):
        b, parity = core // 2, core % 2
        o = res.results[core]["out"]
        for s in range(T // SL_):
            full[b, s * SL_ + parity * HS : s * SL_ + (parity + 1) * HS] = \
                o[s * HS : (s + 1) * HS]
    return full


# revision 20
# speedup vs baseline: 1.4580x; 1.0086x over previous
"""Trainium2 Bass kernel for a dense transformer block (B=4,T=2048,H=16,D=64,C=1024,FF=4096).

Sharding: batch b -> core pair (2b, 2b+1). Within a pair, attention is split by
heads (8 heads/core, Megatron column-parallel QKV + row-parallel W_o), the
attention output partial sums are combined with a pair ReduceScatter, and each
core then runs the full-FF MLP on its half (1024) of the rows.

v2 attention pipeline (per 512-wide q-slice, heads processed in groups of 4):
  - S^T = K @ Q^T per head pair with 64-row PE tiling: the two heads of a pair
    live at SBUF partitions 0-63 / 64-127, so their K=64 matmuls go to PE row
    tiles (0,0)/(64,0) and run concurrently.
  - exp on ScalarE (the attention bottleneck engine) -> PT bf16.
  - softmax denominators via M=1 matmuls (ones_col.T @ PT) 4-way column-tiled
    to PSUM partitions 0/32/64/96 of one bank, accumulated over key chunks.
  - One batched reciprocal per 4-head group, recip broadcast to 64 partitions
    via paired fp32 matmuls (row tiles 0/32/64/96 x col tiles 0/64).
  - AV as V^T @ PT with M=64 column-tiled pairs: head A -> PSUM 0-63, head B
    -> 64-127 of one bank; denominator folded in afterwards by one DVE
    multiply per pair (po * recb) straight into YT.
  - Fully-masked key blocks are never computed; the partially-masked diagonal
    128-block is exp'd then multiplied by a constant triangle (DVE). The
    region left of the diagonal is skipped via subrange accumulation in the
    den/AV matmuls (kc=0 always starts full-width), so no memsets.

LayerNorm affines are folded into the following matmul weights on the host.
Matmuls run in bf16 with fp32 PSUM accumulation. QKV is produced per-512-slice
(Q then K then V) so attention starts while QKV is still streaming; QKV bias
adds run on VectorE to keep ScalarE free for exp. The MLP FC loop runs
t-slice-major (palindrome over weight chunks) so the last ReduceScatter chunk
and LN2 tail hide under the first FC t-slice.
"""

import math

import ml_dtypes
import numpy as np

P = 128
B, T, H, D = 4, 2048, 16, 64
C = H * D
FF = 4096
EPS = 1e-5
N_CORES = 8

_CACHE = {}
LAST_RESULT = None


def _build(T, C, H, D, FF, n_cores, groups, phase_limit=99, sim_safe=False):
    """Build + compile the single-core SPMD program. Returns the Bacc object."""
    from contextlib import ExitStack

    import concourse.mybir as mybir
    import concourse.tile as tile
    from concourse import bacc

    dt = mybir.dt
    AF = mybir.ActivationFunctionType
    OP = mybir.AluOpType

    HH = H // 2               # heads per core
    QH = HH * D               # per-core c_out for each of q,k,v
    NQH = QH // P
    NT = T // P
    T2 = T // 2               # own rows
    NT2 = T2 // P
    NC = C // P
    NF = FF // P
    SL = min(512, T)          # attention q-slice width
    NSL = T // SL
    DBLK = SL // P
    HPC = P // D              # heads per 128-partition chunk (=2)
    FCW = min(512, FF)        # wfc col-chunk width
    TSW = min(512, T)         # qkv t-slice width
    NTS = T // TSW
    CSW = min(512, C)
    NCS = C // CSW
    TS2 = min(512, T2)
    NT2S = T2 // TS2
    NB = 4 if NC % 4 == 0 else 1  # transposes batched per psum bank
    assert QH % P == 0 and T % SL == 0 and SL % P == 0

    nc = bacc.Bacc("TRN2", target_bir_lowering=False, debug=False,
                   num_devices=n_cores)
    gelu_af = (mybir.ActivationFunctionType.Identity if sim_safe
               else mybir.ActivationFunctionType.Gelu)

    # ---- kernel I/O ----
    x_full = nc.dram_tensor("x_full", [T, C], dt.float32, kind="ExternalInput")
    x_own = nc.dram_tensor("x_own", [T2, C], dt.float32, kind="ExternalInput")
    wq = nc.dram_tensor("wq", [C, QH], dt.bfloat16, kind="ExternalInput")
    wk = nc.dram_tensor("wk", [C, QH], dt.bfloat16, kind="ExternalInput")
    wv = nc.dram_tensor("wv", [C, QH], dt.bfloat16, kind="ExternalInput")
    bq = nc.dram_tensor("bq", [QH], dt.float32, kind="ExternalInput")
    bk = nc.dram_tensor("bk", [QH], dt.float32, kind="ExternalInput")
    bv = nc.dram_tensor("bv", [QH], dt.float32, kind="ExternalInput")
    wo = nc.dram_tensor("wo", [QH, C], dt.bfloat16, kind="ExternalInput")
    bo = nc.dram_tensor("bo", [C], dt.float32, kind="ExternalInput")
    wfc = nc.dram_tensor("wfc", [C, FF], dt.bfloat16, kind="ExternalInput")
    bfc = nc.dram_tensor("bfc", [FF], dt.float32, kind="ExternalInput")
    wout = nc.dram_tensor("wout", [FF, C], dt.bfloat16, kind="ExternalInput")
    bout = nc.dram_tensor("bout", [C], dt.float32, kind="ExternalInput")
    tri = nc.dram_tensor("tri", [P, P], dt.bfloat16, kind="ExternalInput")
    ident = nc.dram_tensor("ident", [P, P], dt.bfloat16, kind="ExternalInput")
    out = nc.dram_tensor("out", [T2, C], dt.float32, kind="ExternalOutput")

    # collective bounce buffers (internal DRAM)
    r_bounce = nc.dram_tensor("r_bounce", [T, C], dt.bfloat16)
    r_own_b = nc.dram_tensor("r_own_b", [T2, C], dt.bfloat16)

    x_r = x_full.rearrange("(i p) c -> p i c", p=P)
    xo_r = x_own.rearrange("(i p) c -> p i c", p=P)
    out_r = out.rearrange("(i p) c -> p i c", p=P)
    rb_r = r_bounce.rearrange("(i p) c -> p i c", p=P)
    rob_r = r_own_b.rearrange("(i p) c -> p i c", p=P)

    with tile.TileContext(nc) as tc, ExitStack() as stk:
        pool_const = stk.enter_context(tc.tile_pool(name="const", bufs=1))

        tri_sb = pool_const.tile([P, P], dt.bfloat16)
        id_sb = pool_const.tile([P, P], dt.bfloat16)
        nc.sync.dma_start(tri_sb[:], tri[:])
        nc.sync.dma_start(id_sb[:], ident[:])
        bq_sb = pool_const.tile([P, NQH], dt.float32)
        bk_sb = pool_const.tile([P, NQH], dt.float32)
        bv_row = pool_const.tile([1, QH], dt.float32)
        bo_row = pool_const.tile([1, C], dt.float32)
        bfc_sb = pool_const.tile([P, NF], dt.float32)
        bout_row = pool_const.tile([1, C], dt.float32)
        eps_sb = pool_const.tile([P, 1], dt.float32)
        nc.vector.memset(eps_sb[:], EPS)
        ones1 = pool_const.tile([1, P], dt.float32)
        nc.vector.memset(ones1[:], 1.0)
        bv_full = pool_const.tile([P, QH], dt.bfloat16)
        bo_full = pool_const.tile([P, C], dt.bfloat16)
        bout_full = pool_const.tile([P, C], dt.bfloat16)
        nc.sync.dma_start(bq_sb[:], bq.rearrange("(a p) -> p a", p=P))
        nc.sync.dma_start(bk_sb[:], bk.rearrange("(a p) -> p a", p=P))
        nc.sync.dma_start(bv_row[:], bv[None, :])
        nc.sync.dma_start(bo_row[:], bo[None, :])
        nc.sync.dma_start(bfc_sb[:], bfc.rearrange("(a p) -> p a", p=P))
        nc.sync.dma_start(bout_row[:], bout[None, :])

        def ln_alloc(pool, n_chunks, nm):
            tiles = {}
            for t in ("s1", "s2", "mean", "var", "rstd", "nmr"):
                tiles[t] = pool.tile([P, n_chunks], dt.float32,
                                     tag=f"ln_{t}", name=f"{t}_{nm}")
            return tiles

        def ln_chunk(st, i, xc):
            # stats + rstd/-mean*rstd for one [P, C] chunk (column i)
            ii = slice(i, i + 1)
            nc.vector.reduce_sum(st["s1"][:, ii], xc,
                                 axis=mybir.AxisListType.X)
            nc.vector.tensor_scalar_mul(st["mean"][:, ii], st["s1"][:, ii],
                                        1.0 / C)
            nc.vector.tensor_scalar_mul(st["var"][:, ii], st["s2"][:, ii],
                                        1.0 / C)
            nc.vector.tensor_tensor(st["nmr"][:, ii], st["mean"][:, ii],
                                    st["mean"][:, ii], OP.mult)
            nc.vector.tensor_tensor(st["var"][:, ii], st["var"][:, ii],
                                    st["nmr"][:, ii], OP.subtract)
            nc.scalar.activation(st["var"][:, ii], st["var"][:, ii], AF.Sqrt,
                                 bias=eps_sb[:])
            nc.vector.reciprocal_approx_fast(st["rstd"][:, ii],
                                             st["var"][:, ii])
            nc.vector.tensor_tensor(st["nmr"][:, ii], st["mean"][:, ii],
                                    st["rstd"][:, ii], OP.mult)
            nc.vector.tensor_scalar_mul(st["nmr"][:, ii], st["nmr"][:, ii],
                                        -1.0)

        # attn persistents open first so everything transient frees above them
        with tc.tile_pool(name="pattn", bufs=1) as pool_attn:
            QT = pool_attn.tile([P, NQH, T], dt.bfloat16, tag="QT")
            KT = pool_attn.tile([P, NQH, T], dt.bfloat16, tag="KT")
            V = pool_attn.tile([P, NT, HH, D + 1], dt.bfloat16, tag="V")
            YT = pool_attn.tile([P, NQH, T], dt.bfloat16, tag="YT")
            wo_sb = pool_attn.tile([P, NQH, C], dt.bfloat16, tag="wo")
            nc.vector.memset(V[:, :, :, D], 1.0)

            with ExitStack() as es_zt:
                pool_zt = es_zt.enter_context(tc.tile_pool(name="pzt", bufs=1))
                ZT = pool_zt.tile([P, NC, T], dt.bfloat16)
                pool_wqkv = es_zt.enter_context(tc.tile_pool(name="pw1",
                                                             bufs=1))
                wq_sb = pool_wqkv.tile([P, NC, QH], dt.bfloat16, tag="wq")
                wk_sb = pool_wqkv.tile([P, NC, QH], dt.bfloat16, tag="wk")
                wv_sb = pool_wqkv.tile([P, NC, QH], dt.bfloat16, tag="wv")

                # ===== phase 0: stream x, LN1 stats, z, z^T =====
                with tc.tile_pool(name="pstat", bufs=1) as pool_stat, \
                     tc.tile_pool(name="pxs", bufs=2) as pool_xs, \
                     tc.tile_pool(name="ps_tra", bufs=2, space="PSUM") as ps_tra:
                    for row, full, w in ((bv_row, bv_full, QH),
                                         (bo_row, bo_full, C),
                                         (bout_row, bout_full, C)):
                        for o in range(0, w, 512):
                            wch = min(512, w - o)
                            pb = ps_tra.tile([P, 512], dt.float32, tag="bc")
                            nc.tensor.matmul(pb[:, :wch], ones1[:],
                                             row[:, o : o + wch])
                            nc.vector.tensor_copy(full[:, o : o + wch],
                                                  pb[:, :wch])

                    st1 = ln_alloc(pool_stat, NT, "ln1")
                    GRP = 2 if NT % 2 == 0 else 1
                    xg = None
                    for i in range(NT):
                        if i % GRP == 0:
                            xg = pool_xs.tile([P, GRP, C], dt.float32,
                                              tag="xg")
                            nc.sync.dma_start(
                                xg[:], x_r[:, i : i + GRP, :])
                        xc = xg[:, i % GRP, :]
                        sq = pool_xs.tile([P, C], dt.bfloat16, tag="sq")
                        nc.scalar.activation(sq[:], xc, AF.Square,
                                             accum_out=st1["s2"][:, i : i + 1])
                        ln_chunk(st1, i, xc)
                        zc = pool_xs.tile([P, C], dt.bfloat16, tag="zc")
                        nc.scalar.activation(zc[:], xc, AF.Identity,
                                             bias=st1["nmr"][:, i : i + 1],
                                             scale=st1["rstd"][:, i : i + 1])
                        for jj in range(NC // NB):
                            pt = ps_tra.tile([P, NB * P], dt.bfloat16,
                                             tag="trp")
                            for j4 in range(NB):
                                j = jj * NB + j4
                                nc.tensor.transpose(
                                    pt[:, j4 * P : (j4 + 1) * P],
                                    zc[:, j * P : (j + 1) * P], id_sb[:])
                            nc.vector.tensor_copy(
                                ZT[:, jj * NB : (jj + 1) * NB,
                                   i * P : (i + 1) * P],
                                pt[:].rearrange("p (a b) -> p a b", a=NB))

                # weight DMAs issued after the x stream so the first x
                # tiles are not queued behind 5MB of weights
                nc.sync.dma_start(wq_sb[:],
                                  wq.rearrange("(ci p) o -> p ci o", p=P))
                nc.sync.dma_start(wk_sb[:],
                                  wk.rearrange("(ci p) o -> p ci o", p=P))
                nc.sync.dma_start(wv_sb[:],
                                  wv.rearrange("(ci p) o -> p ci o", p=P))
                nc.sync.dma_start(wo_sb[:],
                                  wo.rearrange("(ci p) o -> p ci o", p=P))

                # ===== fused QKV + attention + W_o + chunked RS =====
                # QKV for t-slice s is emitted just before attention q-slice
                # s; the full-width QKV matmuls are the scheduler's filler
                # for PE gaps while ScalarE works through the exps, and they
                # keep the PE activity monitor warm.
                inv_sqrt_d = 1.0 / math.sqrt(D)
                TPS = SL // P      # t-chunks per q-slice
                with tc.tile_pool(name="ppt", bufs=3) as pool_pt, \
                     tc.tile_pool(name="prec", bufs=2) as pool_rec, \
                     tc.tile_pool(name="prs", bufs=2) as pool_rs, \
                     tc.tile_pool(name="ps_pm", bufs=2, space="PSUM") as ps_pm, \
                     tc.tile_pool(name="ps_qkv", bufs=2, space="PSUM") as ps_qkv, \
                     tc.tile_pool(name="ps_av", bufs=2, space="PSUM") as ps_av, \
                     tc.tile_pool(name="ps_wo", bufs=1, space="PSUM") as ps_wo:
                    for s in range(NSL if phase_limit >= 2 else 0):
                        # --- QKV for t-slice s ---
                        for w_sb, dstT, b_sb in ((wq_sb, QT, bq_sb),
                                                 (wk_sb, KT, bk_sb)):
                            for co in range(NQH):
                                pm = ps_qkv.tile([P, TSW], dt.float32,
                                                 tag="mmp")
                                for ci in range(NC):
                                    nc.tensor.matmul(
                                        pm[:],
                                        w_sb[:, ci, co * P : (co + 1) * P],
                                        ZT[:, ci, s * TSW : (s + 1) * TSW],
                                        start=(ci == 0), stop=(ci == NC - 1))
                                nc.vector.tensor_scalar_add(
                                    dstT[:, co, s * TSW : (s + 1) * TSW],
                                    pm[:], b_sb[:, co : co + 1])
                        for ti in range(s * TPS, (s + 1) * TPS):
                            pm = ps_qkv.tile([P, TSW], dt.float32, tag="mmp")
                            for ci in range(NC):
                                nc.tensor.matmul(
                                    pm[:, :QH],
                                    ZT[:, ci, ti * P : (ti + 1) * P],
                                    wv_sb[:, ci, :],
                                    start=(ci == 0), stop=(ci == NC - 1))
                            nc.vector.tensor_tensor(
                                V[:, ti, :, :D],
                                pm[:, :QH].rearrange("p (h d) -> p h d", d=D),
                                bv_full[:].rearrange("p (h d) -> p h d", d=D),
                                OP.add)

                        # --- attention q-slice s, heads in pairs ---
                        kcm = (s + 1) * DBLK
                        for j in range(NQH):         # pair index == hc chunk
                            PT0 = pool_pt.tile([P, NT, SL], dt.bfloat16,
                                               tag="PT")
                            PT1 = pool_pt.tile([P, NT, SL], dt.bfloat16,
                                               tag="PT")
                            for kk in range(0, kcm, 2):
                                # two key chunks share a 2-bank psum tile and
                                # one exp instruction per head (the masked-off
                                # lead-in of the second band chunk is exp'd as
                                # garbage but never read downstream)
                                c0a = max(kk - s * DBLK, 0) * P
                                c0b = max(kk + 1 - s * DBLK, 0) * P
                                pm0 = ps_pm.tile([P, 2, SL], dt.float32,
                                                 tag="sp")
                                pm1 = ps_pm.tile([P, 2, SL], dt.float32,
                                                 tag="sp")
                                for dk, pmx, qt0 in ((0, pm0, 0), (0, pm1, D),
                                                     (1, pm0, 0), (1, pm1, D)):
                                    kc = kk + dk
                                    c0 = (c0a, c0b)[dk]
                                    nc.tensor.matmul(
                                        pmx[:, dk, c0:],
                                        KT[qt0 : qt0 + D, j,
                                           kc * P : (kc + 1) * P],
                                        QT[qt0 : qt0 + D, j,
                                           s * SL + c0 : (s + 1) * SL],
                                        start=True, stop=True)
                                nc.scalar.activation(
                                    PT0[:, kk : kk + 2, c0a:],
                                    pm0[:, :, c0a:], AF.Exp,
                                    scale=inv_sqrt_d)
                                nc.scalar.activation(
                                    PT1[:, kk : kk + 2, c0a:],
                                    pm1[:, :, c0a:], AF.Exp,
                                    scale=inv_sqrt_d)
                                for dk in range(2):
                                    kc = kk + dk
                                    if kc >= s * DBLK:
                                        c0 = (c0a, c0b)[dk]
                                        nc.vector.tensor_tensor(
                                            PT0[:, kc, c0 : c0 + P],
                                            PT0[:, kc, c0 : c0 + P],
                                            tri_sb[:], OP.mult)
                                        nc.vector.tensor_tensor(
                                            PT1[:, kc, c0 : c0 + P],
                                            PT1[:, kc, c0 : c0 + P],
                                            tri_sb[:], OP.mult)

                            if s >= 1:
                                for _w in range(5 if s < 2 else 8):
                                    dmy = ps_qkv.tile([P, TSW], dt.float32,
                                                      tag="mmp")
                                    nc.tensor.matmul(
                                        dmy[:], id_sb[:],
                                        wq_sb[:, 0, 0:TSW],
                                        start=True, stop=True)

                            for hh, PTh in ((2 * j, PT0), (2 * j + 1, PT1)):
                                hp = D * (hh % 2)
                                po = ps_av.tile([P, SL], dt.float32,
                                                tag="op")
                                for kc in range(kcm):
                                    c0 = max(kc - s * DBLK, 0) * P
                                    nc.tensor.matmul(
                                        po[: D + 1, c0:],
                                        V[:, kc, hh, :],
                                        PTh[:, kc, c0:],
                                        start=(kc == 0),
                                        stop=(kc == kcm - 1))
                                den = pool_rec.tile([1, SL], dt.float32,
                                                    tag="den")
                                recb = pool_rec.tile([D, SL], dt.float32,
                                                     tag="recb")
                                nc.vector.tensor_copy(recb[0:1, :],
                                                      po[D : D + 1, :])
                                nc.vector.reciprocal_approx_fast(
                                    den[:], recb[0:1, :])
                                nc.gpsimd.partition_broadcast(
                                    recb[:], den[:], channels=D)
                                nc.vector.tensor_tensor(
                                    YT[hp : hp + D, j,
                                       s * SL : (s + 1) * SL],
                                    po[:D, :], recb[:], OP.mult)

                        # W_o for this q-slice's t-chunks, then its RS chunk
                        for tis in range(TPS):
                            ti = s * TPS + tis
                            r_sb = pool_rs.tile([P, C], dt.bfloat16,
                                                tag="rsb")
                            for cs in range(NCS):
                                pm = ps_av.tile([P, CSW], dt.float32,
                                                tag="op")
                                for ci in range(NQH):
                                    nc.tensor.matmul(
                                        pm[:],
                                        YT[:, ci, ti * P : (ti + 1) * P],
                                        wo_sb[:, ci, cs * CSW : (cs + 1) * CSW],
                                        start=(ci == 0), stop=(ci == NQH - 1))
                                nc.vector.tensor_copy(
                                    r_sb[:, cs * CSW : (cs + 1) * CSW], pm[:])
                            nc.sync.dma_start(rb_r[:, ti, :], r_sb[:])
                        if phase_limit >= 4:
                            nc.gpsimd.collective_compute(
                                "ReduceScatter", OP.add, replica_groups=groups,
                                ins=[r_bounce[s * SL : (s + 1) * SL, :].opt()],
                                outs=[r_own_b[s * (SL // 2) :
                                              (s + 1) * (SL // 2), :].opt()])

        # ===== phase 4 + 5 =====
        with tc.tile_pool(name="px2", bufs=1) as pool_x2:
            X2 = pool_x2.tile([P, NT2, C], dt.float32, tag="x2")
            st2 = ln_alloc(pool_x2, NT2, "ln2")

            with tc.tile_pool(name="pht", bufs=1) as pool_ht:
                HT = pool_ht.tile([P, NF, T2], dt.bfloat16)

                with ExitStack() as es_z2t:
                    pool_z2t = es_z2t.enter_context(
                        tc.tile_pool(name="pz2t", bufs=1))
                    Z2Ts = [pool_z2t.tile([P, NC, TS2], dt.bfloat16,
                                          tag=f"z2t{k}", name=f"z2t_{k}")
                            for k in range(NT2S)]
                    pool_wfc = es_z2t.enter_context(
                        tc.tile_pool(name="pwfc", bufs=2))
                    ps_h = es_z2t.enter_context(
                        tc.tile_pool(name="ps_h", bufs=4, space="PSUM"))

                    # phase 4: residual + LN2 + z2 + z2^T
                    with tc.tile_pool(name="pxo", bufs=3) as pool_xo, \
                         tc.tile_pool(name="ps_trb", bufs=2,
                                      space="PSUM") as ps_trb:
                        NT2_g = NT2 if phase_limit >= 5 else 0
                        for i in range(NT2_g):
                            for _w in range(3):
                                dmy = ps_h.tile([P, TS2], dt.float32,
                                                tag="hp")
                                nc.tensor.matmul(dmy[:], id_sb[:],
                                                 wo_sb[:, 0, 0:TS2],
                                                 start=True, stop=True)
                            xoc = pool_xo.tile([P, C], dt.float32, tag="xoc")
                            roc = pool_xo.tile([P, C], dt.bfloat16, tag="roc")
                            nc.sync.dma_start(xoc[:], xo_r[:, i, :])
                            nc.sync.dma_start(roc[:], rob_r[:, i, :])
                            nc.vector.tensor_tensor(X2[:, i, :], xoc[:],
                                                    roc[:], OP.add)
                            nc.vector.tensor_tensor(
                                X2[:, i, :], X2[:, i, :], bo_full[:], OP.add)
                            sq = pool_xo.tile([P, C], dt.bfloat16, tag="sq2")
                            nc.scalar.activation(sq[:], X2[:, i, :],
                                                 AF.Square,
                                                 accum_out=st2["s2"][:, i : i + 1])
                            ln_chunk(st2, i, X2[:, i, :])
                            z2c = pool_xo.tile([P, C], dt.bfloat16, tag="z2c")
                            nc.scalar.activation(z2c[:], X2[:, i, :],
                                                 AF.Identity,
                                                 bias=st2["nmr"][:, i : i + 1],
                                                 scale=st2["rstd"][:, i : i + 1])
                            for jj in range(NC // NB):
                                pt = ps_trb.tile([P, NB * P], dt.bfloat16,
                                                 tag="trp")
                                for j4 in range(NB):
                                    j = jj * NB + j4
                                    nc.tensor.transpose(
                                        pt[:, j4 * P : (j4 + 1) * P],
                                        z2c[:, j * P : (j + 1) * P], id_sb[:])
                                nc.vector.tensor_copy(
                                    Z2Ts[(i * P) // TS2][
                                        :, jj * NB : (jj + 1) * NB,
                                        (i * P) % TS2 : (i * P) % TS2 + P],
                                    pt[:].rearrange("p (a b) -> p a b", a=NB))

                    # phase 5a: FC + gelu, t-slice-major with palindrome
                    # weight streaming so the LN2 tail hides under ts_=0
                    if phase_limit >= 6:
                        for ts_ in range(NT2S):
                            fo_order = (range(FF // FCW) if ts_ % 2 == 0
                                        else reversed(range(FF // FCW)))
                            for fo in fo_order:
                                wfc_sb = pool_wfc.tile([P, NC, FCW],
                                                       dt.bfloat16, tag="wfc")
                                nc.sync.dma_start(
                                    wfc_sb[:],
                                    wfc[:, fo * FCW : (fo + 1) * FCW]
                                    .rearrange("(ci p) o -> p ci o", p=P))
                                for f in range(FCW // P):
                                    fg = fo * (FCW // P) + f
                                    pm = ps_h.tile([P, TS2], dt.float32,
                                                   tag="hp")
                                    for ci in range(NC):
                                        nc.tensor.matmul(
                                            pm[:],
                                            wfc_sb[:, ci, f * P : (f + 1) * P],
                                            Z2Ts[ts_][:, ci, :],
                                            start=(ci == 0),
                                            stop=(ci == NC - 1))
                                    nc.scalar.activation(
                                        HT[:, fg, ts_ * TS2 : (ts_ + 1) * TS2],
                                        pm[:], gelu_af,
                                        bias=bfc_sb[:, fg : fg + 1])
                es_z2t.close()

                # phase 5b: W_out + residual
                with tc.tile_pool(name="pwout", bufs=3) as pool_wout, \
                     tc.tile_pool(name="pout", bufs=3) as pool_out, \
                     tc.tile_pool(name="ps_out", bufs=1,
                                  space="PSUM") as ps_out:
                    for cs in range(NCS if phase_limit >= 7 else 0):
                        pms = [ps_out.tile([P, CSW], dt.float32,
                                           tag=f"outp{ti}",
                                           name=f"outp_{cs}_{ti}")
                               for ti in range(NT2)]
                        for fi in range(NF):
                            wout_sb = pool_wout.tile([P, CSW], dt.bfloat16,
                                                     tag="wout")
                            nc.sync.dma_start(
                                wout_sb[:],
                                wout[fi * P : (fi + 1) * P,
                                     cs * CSW : (cs + 1) * CSW])
                            for ti in range(NT2):
                                nc.tensor.matmul(
                                    pms[ti][:],
                                    HT[:, fi, ti * P : (ti + 1) * P],
                                    wout_sb[:],
                                    start=(fi == 0), stop=(fi == NF - 1))
                        for ti in range(NT2):
                            o_sb = pool_out.tile([P, CSW], dt.float32,
                                                 tag="osb")
                            nc.vector.tensor_tensor(
                                o_sb[:], pms[ti][:],
                                X2[:, ti, cs * CSW : (cs + 1) * CSW], OP.add)
                            nc.vector.tensor_tensor(
                                o_sb[:], o_sb[:],
                                bout_full[:, cs * CSW : (cs + 1) * CSW],
                                OP.add)
                            nc.sync.dma_start(
                                out_r[:, ti, cs * CSW : (cs + 1) * CSW],
                                o_sb[:])

    nc.compile()
    return nc


def _prep_core_inputs(b, parity, x, ln1_w, ln1_b, w_qkv, b_qkv, w_o, b_o,
                      ln2_w, ln2_b, w_fc, b_fc, w_out, b_out,
                      T_, C_, H_, D_):
    """Host-side per-core input dict (weights LN-folded, matmul inputs bf16)."""
    bf16 = ml_dtypes.bfloat16
    HH = H_ // 2
    QH = HH * D_
    T2 = T_ // 2
    wq_eff = (ln1_w[:, None] * w_qkv).astype(np.float32)
    bq_eff = (b_qkv + ln1_b @ w_qkv).astype(np.float32)
    wfc_eff = (ln2_w[:, None] * w_fc).astype(np.float32)
    bfc_eff = (b_fc + ln2_b @ w_fc).astype(np.float32)

    h0 = parity * QH
    sl_q = slice(h0, h0 + QH)
    sl_k = slice(C_ + h0, C_ + h0 + QH)
    sl_v = slice(2 * C_ + h0, 2 * C_ + h0 + QH)
    tri = np.tril(np.ones((P, P), np.float32)).T  # tri[k,q] = 1 if k <= q
    ident = np.eye(P, dtype=np.float32)
    SL_ = min(512, T_)
    HS = SL_ // 2
    own_rows = np.concatenate([
        np.arange(s * SL_ + parity * HS, s * SL_ + (parity + 1) * HS)
        for s in range(T_ // SL_)])
    return {
        "x_full": np.ascontiguousarray(x[b]),
        "x_own": np.ascontiguousarray(x[b, own_rows]),
        "wq": np.ascontiguousarray(wq_eff[:, sl_q]).astype(bf16),
        "wk": np.ascontiguousarray(wq_eff[:, sl_k]).astype(bf16),
        "wv": np.ascontiguousarray(wq_eff[:, sl_v]).astype(bf16),
        "bq": np.ascontiguousarray(bq_eff[sl_q]),
        "bk": np.ascontiguousarray(bq_eff[sl_k]),
        "bv": np.ascontiguousarray(bq_eff[sl_v]),
        "wo": np.ascontiguousarray(w_o[h0 : h0 + QH, :]).astype(bf16),
        "bo": np.ascontiguousarray(b_o),
        "wfc": np.ascontiguousarray(wfc_eff).astype(bf16),
        "bfc": np.ascontiguousarray(bfc_eff),
        "wout": np.ascontiguousarray(w_out).astype(bf16),
        "bout": np.ascontiguousarray(b_out),
        "tri": tri.astype(bf16),
        "ident": ident.astype(bf16),
    }


def kernel(x, ln1_w, ln1_b, w_qkv, b_qkv, w_o, b_o, ln2_w, ln2_b,
           w_fc, b_fc, w_out, b_out):
    from concourse.bass_utils import run_bass_kernel_spmd

    key = (T, C, H, D, FF, N_CORES)
    if key not in _CACHE:
        groups = [[2 * i, 2 * i + 1] for i in range(N_CORES // 2)]
        _CACHE[key] = _build(T, C, H, D, FF, N_CORES, groups)
    nc = _CACHE[key]

    args = (np.asarray(x, np.float32), np.asarray(ln1_w, np.float32),
            np.asarray(ln1_b, np.float32), np.asarray(w_qkv, np.float32),
            np.asarray(b_qkv, np.float32), np.asarray(w_o, np.float32),
            np.asarray(b_o, np.float32), np.asarray(ln2_w, np.float32),
            np.asarray(ln2_b, np.float32), np.asarray(w_fc, np.float32),
            np.asarray(b_fc, np.float32), np.asarray(w_out, np.float32),
            np.asarray(b_out, np.float32))
    in_maps = []
    for core in range(N_CORES):
        b, parity = core // 2, core % 2
        in_maps.append(_prep_core_inputs(b, parity, *args, T, C, H, D))

    global LAST_RESULT
    res = run_bass_kernel_spmd(nc, in_maps, core_ids=list(range(N_CORES)))
    LAST_RESULT = res

    SL_ = min(512, T)
    HS = SL_ // 2
    full = np.empty((B, T, C), np.float32)
    for core in range(N_CORES):
        b, parity = core // 2, core % 2
        o = res.results[core]["out"]
        for s in range(T // SL_):
            full[b, s * SL_ + parity * HS : s * SL_ + (parity + 1) * HS] = \
                o[s * HS : (s + 1) * HS]
    return full


# revision 21
# speedup vs baseline: 1.4920x; 1.0233x over previous
"""Trainium2 Bass kernel for a dense transformer block (B=4,T=2048,H=16,D=64,C=1024,FF=4096).

Sharding: batch b -> core pair (2b, 2b+1). Within a pair, attention is split by
heads (8 heads/core, Megatron column-parallel QKV + row-parallel W_o), the
attention output partial sums are combined with a pair ReduceScatter, and each
core then runs the full-FF MLP on its half (1024) of the rows.

v2 attention pipeline (per 512-wide q-slice, heads processed in groups of 4):
  - S^T = K @ Q^T per head pair with 64-row PE tiling: the two heads of a pair
    live at SBUF partitions 0-63 / 64-127, so their K=64 matmuls go to PE row
    tiles (0,0)/(64,0) and run concurrently.
  - exp on ScalarE (the attention bottleneck engine) -> PT bf16.
  - softmax denominators via M=1 matmuls (ones_col.T @ PT) 4-way column-tiled
    to PSUM partitions 0/32/64/96 of one bank, accumulated over key chunks.
  - One batched reciprocal per 4-head group, recip broadcast to 64 partitions
    via paired fp32 matmuls (row tiles 0/32/64/96 x col tiles 0/64).
  - AV as V^T @ PT with M=64 column-tiled pairs: head A -> PSUM 0-63, head B
    -> 64-127 of one bank; denominator folded in afterwards by one DVE
    multiply per pair (po * recb) straight into YT.
  - Fully-masked key blocks are never computed; the partially-masked diagonal
    128-block is exp'd then multiplied by a constant triangle (DVE). The
    region left of the diagonal is skipped via subrange accumulation in the
    den/AV matmuls (kc=0 always starts full-width), so no memsets.

LayerNorm affines are folded into the following matmul weights on the host.
Matmuls run in bf16 with fp32 PSUM accumulation. QKV is produced per-512-slice
(Q then K then V) so attention starts while QKV is still streaming; QKV bias
adds run on VectorE to keep ScalarE free for exp. The MLP FC loop runs
t-slice-major (palindrome over weight chunks) so the last ReduceScatter chunk
and LN2 tail hide under the first FC t-slice.
"""

import math

import ml_dtypes
import numpy as np

P = 128
B, T, H, D = 4, 2048, 16, 64
C = H * D
FF = 4096
EPS = 1e-5
N_CORES = 8

_CACHE = {}
LAST_RESULT = None


def _build(T, C, H, D, FF, n_cores, groups, phase_limit=99, sim_safe=False):
    """Build + compile the single-core SPMD program. Returns the Bacc object."""
    from contextlib import ExitStack

    import concourse.mybir as mybir
    import concourse.tile as tile
    from concourse import bacc

    dt = mybir.dt
    AF = mybir.ActivationFunctionType
    OP = mybir.AluOpType

    HH = H // 2               # heads per core
    QH = HH * D               # per-core c_out for each of q,k,v
    NQH = QH // P
    NT = T // P
    T2 = T // 2               # own rows
    NT2 = T2 // P
    NC = C // P
    NF = FF // P
    SL = min(512, T)          # attention q-slice width
    NSL = T // SL
    DBLK = SL // P
    HPC = P // D              # heads per 128-partition chunk (=2)
    FCW = min(512, FF)        # wfc col-chunk width
    TSW = min(512, T)         # qkv t-slice width
    NTS = T // TSW
    CSW = min(512, C)
    NCS = C // CSW
    TS2 = min(512, T2)
    NT2S = T2 // TS2
    NB = 4 if NC % 4 == 0 else 1  # transposes batched per psum bank
    assert QH % P == 0 and T % SL == 0 and SL % P == 0

    nc = bacc.Bacc("TRN2", target_bir_lowering=False, debug=False,
                   num_devices=n_cores)
    gelu_af = (mybir.ActivationFunctionType.Identity if sim_safe
               else mybir.ActivationFunctionType.Gelu)

    # ---- kernel I/O ----
    x_full = nc.dram_tensor("x_full", [T, C], dt.float32, kind="ExternalInput")
    x_own = nc.dram_tensor("x_own", [T2, C], dt.float32, kind="ExternalInput")
    wq = nc.dram_tensor("wq", [C, QH], dt.bfloat16, kind="ExternalInput")
    wk = nc.dram_tensor("wk", [C, QH], dt.bfloat16, kind="ExternalInput")
    wv = nc.dram_tensor("wv", [C, QH], dt.bfloat16, kind="ExternalInput")
    bq = nc.dram_tensor("bq", [QH], dt.float32, kind="ExternalInput")
    bk = nc.dram_tensor("bk", [QH], dt.float32, kind="ExternalInput")
    bv = nc.dram_tensor("bv", [QH], dt.float32, kind="ExternalInput")
    wo = nc.dram_tensor("wo", [QH, C], dt.bfloat16, kind="ExternalInput")
    bo = nc.dram_tensor("bo", [C], dt.float32, kind="ExternalInput")
    wfc = nc.dram_tensor("wfc", [C, FF], dt.bfloat16, kind="ExternalInput")
    bfc = nc.dram_tensor("bfc", [FF], dt.float32, kind="ExternalInput")
    wout = nc.dram_tensor("wout", [FF, C], dt.bfloat16, kind="ExternalInput")
    bout = nc.dram_tensor("bout", [C], dt.float32, kind="ExternalInput")
    tri = nc.dram_tensor("tri", [P, P], dt.bfloat16, kind="ExternalInput")
    ident = nc.dram_tensor("ident", [P, P], dt.bfloat16, kind="ExternalInput")
    out = nc.dram_tensor("out", [T2, C], dt.float32, kind="ExternalOutput")

    # collective bounce buffers (internal DRAM)
    r_bounce = nc.dram_tensor("r_bounce", [T, C], dt.bfloat16)
    r_own_b = nc.dram_tensor("r_own_b", [T2, C], dt.bfloat16)

    x_r = x_full.rearrange("(i p) c -> p i c", p=P)
    xo_r = x_own.rearrange("(i p) c -> p i c", p=P)
    out_r = out.rearrange("(i p) c -> p i c", p=P)
    rb_r = r_bounce.rearrange("(i p) c -> p i c", p=P)
    rob_r = r_own_b.rearrange("(i p) c -> p i c", p=P)

    with tile.TileContext(nc) as tc, ExitStack() as stk:
        pool_const = stk.enter_context(tc.tile_pool(name="const", bufs=1))

        tri_sb = pool_const.tile([P, P], dt.bfloat16)
        id_sb = pool_const.tile([P, P], dt.bfloat16)
        nc.sync.dma_start(tri_sb[:], tri[:])
        nc.sync.dma_start(id_sb[:], ident[:])
        bq_sb = pool_const.tile([P, NQH], dt.float32)
        bk_sb = pool_const.tile([P, NQH], dt.float32)
        bv_row = pool_const.tile([1, QH], dt.float32)
        bo_row = pool_const.tile([1, C], dt.float32)
        bfc_sb = pool_const.tile([P, NF], dt.float32)
        bout_row = pool_const.tile([1, C], dt.float32)
        eps_sb = pool_const.tile([P, 1], dt.float32)
        nc.vector.memset(eps_sb[:], EPS)
        ones1 = pool_const.tile([1, P], dt.float32)
        nc.vector.memset(ones1[:], 1.0)
        ones1b = pool_const.tile([1, P], dt.bfloat16)
        nc.vector.memset(ones1b[:], 1.0)
        bout_bf = pool_const.tile([1, C], dt.bfloat16)
        bv_full = pool_const.tile([P, QH], dt.bfloat16)
        bo_full = pool_const.tile([P, C], dt.bfloat16)
        bout_full = pool_const.tile([P, C], dt.bfloat16)
        nc.sync.dma_start(bq_sb[:], bq.rearrange("(a p) -> p a", p=P))
        nc.sync.dma_start(bk_sb[:], bk.rearrange("(a p) -> p a", p=P))
        nc.sync.dma_start(bv_row[:], bv[None, :])
        nc.sync.dma_start(bo_row[:], bo[None, :])
        nc.sync.dma_start(bfc_sb[:], bfc.rearrange("(a p) -> p a", p=P))
        nc.sync.dma_start(bout_row[:], bout[None, :])

        def ln_alloc(pool, n_chunks, nm):
            tiles = {}
            for t in ("s1", "s2", "mean", "var", "rstd", "nmr"):
                tiles[t] = pool.tile([P, n_chunks], dt.float32,
                                     tag=f"ln_{t}", name=f"{t}_{nm}")
            return tiles

        def ln_chunk(st, i, xc):
            # stats + rstd/-mean*rstd for one [P, C] chunk (column i)
            ii = slice(i, i + 1)
            nc.vector.reduce_sum(st["s1"][:, ii], xc,
                                 axis=mybir.AxisListType.X)
            nc.vector.tensor_scalar_mul(st["mean"][:, ii], st["s1"][:, ii],
                                        1.0 / C)
            nc.vector.tensor_scalar_mul(st["var"][:, ii], st["s2"][:, ii],
                                        1.0 / C)
            nc.vector.tensor_tensor(st["nmr"][:, ii], st["mean"][:, ii],
                                    st["mean"][:, ii], OP.mult)
            nc.vector.tensor_tensor(st["var"][:, ii], st["var"][:, ii],
                                    st["nmr"][:, ii], OP.subtract)
            nc.scalar.activation(st["var"][:, ii], st["var"][:, ii], AF.Sqrt,
                                 bias=eps_sb[:])
            nc.vector.reciprocal_approx_fast(st["rstd"][:, ii],
                                             st["var"][:, ii])
            nc.vector.tensor_tensor(st["nmr"][:, ii], st["mean"][:, ii],
                                    st["rstd"][:, ii], OP.mult)
            nc.vector.tensor_scalar_mul(st["nmr"][:, ii], st["nmr"][:, ii],
                                        -1.0)

        # attn persistents open first so everything transient frees above them
        with tc.tile_pool(name="pattn", bufs=1) as pool_attn:
            QT = pool_attn.tile([P, NQH, T], dt.bfloat16, tag="QT")
            KT = pool_attn.tile([P, NQH, T], dt.bfloat16, tag="KT")
            V = pool_attn.tile([P, NT, HH, D + 1], dt.bfloat16, tag="V")
            YT = pool_attn.tile([P, NQH, T], dt.bfloat16, tag="YT")
            wo_sb = pool_attn.tile([P, NQH, C], dt.bfloat16, tag="wo")
            nc.vector.memset(V[:, :, :, D], 1.0)

            with ExitStack() as es_zt:
                pool_zt = es_zt.enter_context(tc.tile_pool(name="pzt", bufs=1))
                ZT = pool_zt.tile([P, NC, T], dt.bfloat16)
                pool_wqkv = es_zt.enter_context(tc.tile_pool(name="pw1",
                                                             bufs=1))
                wq_sb = pool_wqkv.tile([P, NC, QH], dt.bfloat16, tag="wq")
                wk_sb = pool_wqkv.tile([P, NC, QH], dt.bfloat16, tag="wk")
                wv_sb = pool_wqkv.tile([P, NC, QH], dt.bfloat16, tag="wv")

                # ===== phase 0: stream x, LN1 stats, z, z^T =====
                with tc.tile_pool(name="pstat", bufs=1) as pool_stat, \
                     tc.tile_pool(name="pxs", bufs=2) as pool_xs, \
                     tc.tile_pool(name="ps_tra", bufs=2, space="PSUM") as ps_tra:
                    nc.vector.tensor_copy(bout_bf[:], bout_row[:])
                    for row, full, w in ((bv_row, bv_full, QH),
                                         (bo_row, bo_full, C)):
                        for o in range(0, w, 512):
                            wch = min(512, w - o)
                            pb = ps_tra.tile([P, 512], dt.float32, tag="bc")
                            nc.tensor.matmul(pb[:, :wch], ones1[:],
                                             row[:, o : o + wch])
                            nc.vector.tensor_copy(full[:, o : o + wch],
                                                  pb[:, :wch])

                    st1 = ln_alloc(pool_stat, NT, "ln1")
                    GRP = 2 if NT % 2 == 0 else 1
                    xg = None
                    for i in range(NT):
                        if i % GRP == 0:
                            xg = pool_xs.tile([P, GRP, C], dt.float32,
                                              tag="xg")
                            nc.sync.dma_start(
                                xg[:], x_r[:, i : i + GRP, :])
                        xc = xg[:, i % GRP, :]
                        sq = pool_xs.tile([P, C], dt.bfloat16, tag="sq")
                        nc.scalar.activation(sq[:], xc, AF.Square,
                                             accum_out=st1["s2"][:, i : i + 1])
                        ln_chunk(st1, i, xc)
                        zc = pool_xs.tile([P, C], dt.bfloat16, tag="zc")
                        nc.scalar.activation(zc[:], xc, AF.Identity,
                                             bias=st1["nmr"][:, i : i + 1],
                                             scale=st1["rstd"][:, i : i + 1])
                        for jj in range(NC // NB):
                            pt = ps_tra.tile([P, NB * P], dt.bfloat16,
                                             tag="trp")
                            for j4 in range(NB):
                                j = jj * NB + j4
                                nc.tensor.transpose(
                                    pt[:, j4 * P : (j4 + 1) * P],
                                    zc[:, j * P : (j + 1) * P], id_sb[:])
                            nc.vector.tensor_copy(
                                ZT[:, jj * NB : (jj + 1) * NB,
                                   i * P : (i + 1) * P],
                                pt[:].rearrange("p (a b) -> p a b", a=NB))

                # weight DMAs issued after the x stream so the first x
                # tiles are not queued behind 5MB of weights
                nc.sync.dma_start(wq_sb[:],
                                  wq.rearrange("(ci p) o -> p ci o", p=P))
                nc.sync.dma_start(wk_sb[:],
                                  wk.rearrange("(ci p) o -> p ci o", p=P))
                nc.sync.dma_start(wv_sb[:],
                                  wv.rearrange("(ci p) o -> p ci o", p=P))
                nc.sync.dma_start(wo_sb[:],
                                  wo.rearrange("(ci p) o -> p ci o", p=P))

                # ===== fused QKV + attention + W_o + chunked RS =====
                # QKV for t-slice s is emitted just before attention q-slice
                # s; the full-width QKV matmuls are the scheduler's filler
                # for PE gaps while ScalarE works through the exps, and they
                # keep the PE activity monitor warm.
                inv_sqrt_d = 1.0 / math.sqrt(D)
                TPS = SL // P      # t-chunks per q-slice
                with tc.tile_pool(name="ppt", bufs=3) as pool_pt, \
                     tc.tile_pool(name="prec", bufs=2) as pool_rec, \
                     tc.tile_pool(name="prs", bufs=2) as pool_rs, \
                     tc.tile_pool(name="ps_pm", bufs=2, space="PSUM") as ps_pm, \
                     tc.tile_pool(name="ps_qkv", bufs=2, space="PSUM") as ps_qkv, \
                     tc.tile_pool(name="ps_av", bufs=2, space="PSUM") as ps_av, \
                     tc.tile_pool(name="ps_wo", bufs=1, space="PSUM") as ps_wo:
                    for s in range(NSL if phase_limit >= 2 else 0):
                        # --- QKV for t-slice s ---
                        for w_sb, dstT, b_sb in ((wq_sb, QT, bq_sb),
                                                 (wk_sb, KT, bk_sb)):
                            for co in range(NQH):
                                pm = ps_qkv.tile([P, TSW], dt.float32,
                                                 tag="mmp")
                                for ci in range(NC):
                                    nc.tensor.matmul(
                                        pm[:],
                                        w_sb[:, ci, co * P : (co + 1) * P],
                                        ZT[:, ci, s * TSW : (s + 1) * TSW],
                                        start=(ci == 0), stop=(ci == NC - 1))
                                nc.vector.tensor_scalar_add(
                                    dstT[:, co, s * TSW : (s + 1) * TSW],
                                    pm[:], b_sb[:, co : co + 1])
                        for ti in range(s * TPS, (s + 1) * TPS):
                            pm = ps_qkv.tile([P, TSW], dt.float32, tag="mmp")
                            for ci in range(NC):
                                nc.tensor.matmul(
                                    pm[:, :QH],
                                    ZT[:, ci, ti * P : (ti + 1) * P],
                                    wv_sb[:, ci, :],
                                    start=(ci == 0), stop=(ci == NC - 1))
                            nc.vector.tensor_tensor(
                                V[:, ti, :, :D],
                                pm[:, :QH].rearrange("p (h d) -> p h d", d=D),
                                bv_full[:].rearrange("p (h d) -> p h d", d=D),
                                OP.add)

                        # --- attention q-slice s, heads in pairs ---
                        kcm = (s + 1) * DBLK
                        for j in range(NQH):         # pair index == hc chunk
                            PT0 = pool_pt.tile([P, NT, SL], dt.bfloat16,
                                               tag="PT")
                            PT1 = pool_pt.tile([P, NT, SL], dt.bfloat16,
                                               tag="PT")
                            for kk in range(0, kcm, 2):
                                # two key chunks share a 2-bank psum tile and
                                # one exp instruction per head (the masked-off
                                # lead-in of the second band chunk is exp'd as
                                # garbage but never read downstream)
                                c0a = max(kk - s * DBLK, 0) * P
                                c0b = max(kk + 1 - s * DBLK, 0) * P
                                pm0 = ps_pm.tile([P, 2, SL], dt.float32,
                                                 tag="sp")
                                pm1 = ps_pm.tile([P, 2, SL], dt.float32,
                                                 tag="sp")
                                for dk, pmx, qt0 in ((0, pm0, 0), (0, pm1, D),
                                                     (1, pm0, 0), (1, pm1, D)):
                                    kc = kk + dk
                                    c0 = (c0a, c0b)[dk]
                                    nc.tensor.matmul(
                                        pmx[:, dk, c0:],
                                        KT[qt0 : qt0 + D, j,
                                           kc * P : (kc + 1) * P],
                                        QT[qt0 : qt0 + D, j,
                                           s * SL + c0 : (s + 1) * SL],
                                        start=True, stop=True)
                                nc.scalar.activation(
                                    PT0[:, kk : kk + 2, c0a:],
                                    pm0[:, :, c0a:], AF.Exp,
                                    scale=inv_sqrt_d)
                                nc.scalar.activation(
                                    PT1[:, kk : kk + 2, c0a:],
                                    pm1[:, :, c0a:], AF.Exp,
                                    scale=inv_sqrt_d)
                                for dk in range(2):
                                    kc = kk + dk
                                    if kc >= s * DBLK:
                                        c0 = (c0a, c0b)[dk]
                                        nc.vector.tensor_tensor(
                                            PT0[:, kc, c0 : c0 + P],
                                            PT0[:, kc, c0 : c0 + P],
                                            tri_sb[:], OP.mult)
                                        nc.vector.tensor_tensor(
                                            PT1[:, kc, c0 : c0 + P],
                                            PT1[:, kc, c0 : c0 + P],
                                            tri_sb[:], OP.mult)

                            if s >= 1:
                                for _w in range(3 if s < 2 else 8):
                                    dmy = ps_qkv.tile([P, TSW], dt.float32,
                                                      tag="mmp")
                                    nc.tensor.matmul(
                                        dmy[:], id_sb[:],
                                        wq_sb[:, 0, 0:TSW],
                                        start=True, stop=True)

                            for hh, PTh in ((2 * j, PT0), (2 * j + 1, PT1)):
                                hp = D * (hh % 2)
                                po = ps_av.tile([P, SL], dt.float32,
                                                tag="op")
                                for kc in range(kcm):
                                    c0 = max(kc - s * DBLK, 0) * P
                                    nc.tensor.matmul(
                                        po[: D + 1, c0:],
                                        V[:, kc, hh, :],
                                        PTh[:, kc, c0:],
                                        start=(kc == 0),
                                        stop=(kc == kcm - 1))
                                den = pool_rec.tile([1, SL], dt.float32,
                                                    tag="den")
                                recb = pool_rec.tile([D, SL], dt.float32,
                                                     tag="recb")
                                nc.vector.tensor_copy(recb[0:1, :],
                                                      po[D : D + 1, :])
                                nc.vector.reciprocal_approx_fast(
                                    den[:], recb[0:1, :])
                                nc.gpsimd.partition_broadcast(
                                    recb[:], den[:], channels=D)
                                nc.vector.tensor_tensor(
                                    YT[hp : hp + D, j,
                                       s * SL : (s + 1) * SL],
                                    po[:D, :], recb[:], OP.mult)

                        # W_o for this q-slice's t-chunks, then its RS chunk
                        for tis in range(TPS):
                            ti = s * TPS + tis
                            r_sb = pool_rs.tile([P, C], dt.bfloat16,
                                                tag="rsb")
                            for cs in range(NCS):
                                pm = ps_av.tile([P, CSW], dt.float32,
                                                tag="op")
                                for ci in range(NQH):
                                    nc.tensor.matmul(
                                        pm[:],
                                        YT[:, ci, ti * P : (ti + 1) * P],
                                        wo_sb[:, ci, cs * CSW : (cs + 1) * CSW],
                                        start=(ci == 0), stop=(ci == NQH - 1))
                                nc.vector.tensor_copy(
                                    r_sb[:, cs * CSW : (cs + 1) * CSW], pm[:])
                            nc.sync.dma_start(rb_r[:, ti, :], r_sb[:])
                        if phase_limit >= 4:
                            nc.gpsimd.collective_compute(
                                "ReduceScatter", OP.add, replica_groups=groups,
                                ins=[r_bounce[s * SL : (s + 1) * SL, :].opt()],
                                outs=[r_own_b[s * (SL // 2) :
                                              (s + 1) * (SL // 2), :].opt()])

        # ===== phase 4 + 5 =====
        with tc.tile_pool(name="px2", bufs=1) as pool_x2:
            X2 = pool_x2.tile([P, NT2, C], dt.float32, tag="x2")
            st2 = ln_alloc(pool_x2, NT2, "ln2")

            with tc.tile_pool(name="pht", bufs=1) as pool_ht:
                HT = pool_ht.tile([P, NF, T2], dt.bfloat16)

                with ExitStack() as es_z2t:
                    pool_z2t = es_z2t.enter_context(
                        tc.tile_pool(name="pz2t", bufs=1))
                    Z2Ts = [pool_z2t.tile([P, NC, TS2], dt.bfloat16,
                                          tag=f"z2t{k}", name=f"z2t_{k}")
                            for k in range(NT2S)]
                    pool_wfc = es_z2t.enter_context(
                        tc.tile_pool(name="pwfc", bufs=2))
                    ps_h = es_z2t.enter_context(
                        tc.tile_pool(name="ps_h", bufs=4, space="PSUM"))

                    # phase 4: residual + LN2 + z2 + z2^T
                    with tc.tile_pool(name="pxo", bufs=3) as pool_xo, \
                         tc.tile_pool(name="ps_trb", bufs=2,
                                      space="PSUM") as ps_trb:
                        NT2_g = NT2 if phase_limit >= 5 else 0
                        for i in range(NT2_g):
                            for _w in range(3):
                                dmy = ps_h.tile([P, TS2], dt.float32,
                                                tag="hp")
                                nc.tensor.matmul(dmy[:], id_sb[:],
                                                 wo_sb[:, 0, 0:TS2],
                                                 start=True, stop=True)
                            xoc = pool_xo.tile([P, C], dt.float32, tag="xoc")
                            roc = pool_xo.tile([P, C], dt.bfloat16, tag="roc")
                            nc.sync.dma_start(xoc[:], xo_r[:, i, :])
                            nc.sync.dma_start(roc[:], rob_r[:, i, :])
                            nc.vector.tensor_tensor(X2[:, i, :], xoc[:],
                                                    roc[:], OP.add)
                            nc.vector.tensor_tensor(
                                X2[:, i, :], X2[:, i, :], bo_full[:], OP.add)
                            sq = pool_xo.tile([P, C], dt.bfloat16, tag="sq2")
                            nc.scalar.activation(sq[:], X2[:, i, :],
                                                 AF.Square,
                                                 accum_out=st2["s2"][:, i : i + 1])
                            ln_chunk(st2, i, X2[:, i, :])
                            z2c = pool_xo.tile([P, C], dt.bfloat16, tag="z2c")
                            nc.scalar.activation(z2c[:], X2[:, i, :],
                                                 AF.Identity,
                                                 bias=st2["nmr"][:, i : i + 1],
                                                 scale=st2["rstd"][:, i : i + 1])
                            for jj in range(NC // NB):
                                pt = ps_trb.tile([P, NB * P], dt.bfloat16,
                                                 tag="trp")
                                for j4 in range(NB):
                                    j = jj * NB + j4
                                    nc.tensor.transpose(
                                        pt[:, j4 * P : (j4 + 1) * P],
                                        z2c[:, j * P : (j + 1) * P], id_sb[:])
                                nc.vector.tensor_copy(
                                    Z2Ts[(i * P) // TS2][
                                        :, jj * NB : (jj + 1) * NB,
                                        (i * P) % TS2 : (i * P) % TS2 + P],
                                    pt[:].rearrange("p (a b) -> p a b", a=NB))

                    # phase 5a: FC + gelu, t-slice-major with palindrome
                    # weight streaming so the LN2 tail hides under ts_=0
                    if phase_limit >= 6:
                        for ts_ in range(NT2S):
                            fo_order = (range(FF // FCW) if ts_ % 2 == 0
                                        else reversed(range(FF // FCW)))
                            for fo in fo_order:
                                wfc_sb = pool_wfc.tile([P, NC, FCW],
                                                       dt.bfloat16, tag="wfc")
                                nc.sync.dma_start(
                                    wfc_sb[:],
                                    wfc[:, fo * FCW : (fo + 1) * FCW]
                                    .rearrange("(ci p) o -> p ci o", p=P))
                                for f in range(FCW // P):
                                    fg = fo * (FCW // P) + f
                                    pm = ps_h.tile([P, TS2], dt.float32,
                                                   tag="hp")
                                    for ci in range(NC):
                                        nc.tensor.matmul(
                                            pm[:],
                                            wfc_sb[:, ci, f * P : (f + 1) * P],
                                            Z2Ts[ts_][:, ci, :],
                                            start=(ci == 0),
                                            stop=(ci == NC - 1))
                                    nc.scalar.activation(
                                        HT[:, fg, ts_ * TS2 : (ts_ + 1) * TS2],
                                        pm[:], gelu_af,
                                        bias=bfc_sb[:, fg : fg + 1])
                es_z2t.close()

                # phase 5b: W_out + residual
                with tc.tile_pool(name="pwout", bufs=3) as pool_wout, \
                     tc.tile_pool(name="pout", bufs=3) as pool_out, \
                     tc.tile_pool(name="ps_out", bufs=1,
                                  space="PSUM") as ps_out:
                    for cs in range(NCS if phase_limit >= 7 else 0):
                        pms = [ps_out.tile([P, CSW], dt.float32,
                                           tag=f"outp{ti}",
                                           name=f"outp_{cs}_{ti}")
                               for ti in range(NT2)]
                        for ti in range(NT2):
                            nc.tensor.matmul(
                                pms[ti][:], ones1b[:],
                                bout_bf[:, cs * CSW : (cs + 1) * CSW],
                                start=True, stop=False)
                        for fi in range(NF):
                            wout_sb = pool_wout.tile([P, CSW], dt.bfloat16,
                                                     tag="wout")
                            nc.sync.dma_start(
                                wout_sb[:],
                                wout[fi * P : (fi + 1) * P,
                                     cs * CSW : (cs + 1) * CSW])
                            for ti in range(NT2):
                                nc.tensor.matmul(
                                    pms[ti][:],
                                    HT[:, fi, ti * P : (ti + 1) * P],
                                    wout_sb[:],
                                    start=False, stop=(fi == NF - 1))
                        for ti in range(NT2):
                            o_sb = pool_out.tile([P, CSW], dt.float32,
                                                 tag="osb")
                            nc.vector.tensor_tensor(
                                o_sb[:], pms[ti][:],
                                X2[:, ti, cs * CSW : (cs + 1) * CSW], OP.add)
                            nc.sync.dma_start(
                                out_r[:, ti, cs * CSW : (cs + 1) * CSW],
                                o_sb[:])

    nc.compile()
    return nc


def _prep_core_inputs(b, parity, x, ln1_w, ln1_b, w_qkv, b_qkv, w_o, b_o,
                      ln2_w, ln2_b, w_fc, b_fc, w_out, b_out,
                      T_, C_, H_, D_):
    """Host-side per-core input dict (weights LN-folded, matmul inputs bf16)."""
    bf16 = ml_dtypes.bfloat16
    HH = H_ // 2
    QH = HH * D_
    T2 = T_ // 2
    wq_eff = (ln1_w[:, None] * w_qkv).astype(np.float32)
    bq_eff = (b_qkv + ln1_b @ w_qkv).astype(np.float32)
    wfc_eff = (ln2_w[:, None] * w_fc).astype(np.float32)
    bfc_eff = (b_fc + ln2_b @ w_fc).astype(np.float32)

    h0 = parity * QH
    sl_q = slice(h0, h0 + QH)
    sl_k = slice(C_ + h0, C_ + h0 + QH)
    sl_v = slice(2 * C_ + h0, 2 * C_ + h0 + QH)
    tri = np.tril(np.ones((P, P), np.float32)).T  # tri[k,q] = 1 if k <= q
    ident = np.eye(P, dtype=np.float32)
    SL_ = min(512, T_)
    HS = SL_ // 2
    own_rows = np.concatenate([
        np.arange(s * SL_ + parity * HS, s * SL_ + (parity + 1) * HS)
        for s in range(T_ // SL_)])
    return {
        "x_full": np.ascontiguousarray(x[b]),
        "x_own": np.ascontiguousarray(x[b, own_rows]),
        "wq": np.ascontiguousarray(wq_eff[:, sl_q]).astype(bf16),
        "wk": np.ascontiguousarray(wq_eff[:, sl_k]).astype(bf16),
        "wv": np.ascontiguousarray(wq_eff[:, sl_v]).astype(bf16),
        "bq": np.ascontiguousarray(bq_eff[sl_q]),
        "bk": np.ascontiguousarray(bq_eff[sl_k]),
        "bv": np.ascontiguousarray(bq_eff[sl_v]),
        "wo": np.ascontiguousarray(w_o[h0 : h0 + QH, :]).astype(bf16),
        "bo": np.ascontiguousarray(b_o),
        "wfc": np.ascontiguousarray(wfc_eff).astype(bf16),
        "bfc": np.ascontiguousarray(bfc_eff),
        "wout": np.ascontiguousarray(w_out).astype(bf16),
        "bout": np.ascontiguousarray(b_out),
        "tri": tri.astype(bf16),
        "ident": ident.astype(bf16),
    }


def kernel(x, ln1_w, ln1_b, w_qkv, b_qkv, w_o, b_o, ln2_w, ln2_b,
           w_fc, b_fc, w_out, b_out):
    from concourse.bass_utils import run_bass_kernel_spmd

    key = (T, C, H, D, FF, N_CORES)
    if key not in _CACHE:
        groups = [[2 * i, 2 * i + 1] for i in range(N_CORES // 2)]
        _CACHE[key] = _build(T, C, H, D, FF, N_CORES, groups)
    nc = _CACHE[key]

    args = (np.asarray(x, np.float32), np.asarray(ln1_w, np.float32),
            np.asarray(ln1_b, np.float32), np.asarray(w_qkv, np.float32),
            np.asarray(b_qkv, np.float32), np.asarray(w_o, np.float32),
            np.asarray(b_o, np.float32), np.asarray(ln2_w, np.float32),
            np.asarray(ln2_b, np.float32), np.asarray(w_fc, np.float32),
            np.asarray(b_fc, np.float32), np.asarray(w_out, np.float32),
            np.asarray(b_out, np.float32))
    in_maps = []
    for core in range(N_CORES):
        b, parity = core // 2, core % 2
        in_maps.append(_prep_core_inputs(b, parity, *args, T, C, H, D))

    global LAST_RESULT
    res = run_bass_kernel_spmd(nc, in_maps, core_ids=list(range(N_CORES)))
    LAST_RESULT = res

    SL_ = min(512, T)
    HS = SL_ // 2
    full = np.empty((B, T, C), np.float32)
    for core in range(N_CORES):
        b, parity = core // 2, core % 2
        o = res.results[core]["out"]
        for s in range(T // SL_):
            full[b, s * SL_ + parity * HS : s * SL_ + (parity + 1) * HS] = \
                o[s * HS : (s + 1) * HS]
    return full
